# revision 62
# baseline (speedup 1.0000x reference)
"""Bass/Trainium2 kernel for MultiHeadAttentionWithDSA (sparse attention with
lightning-indexer top-64 key selection), sharded over 8 NeuronCores.

Sharding: core = b*4 + g  (b in {0,1} batch, g in {0..3} head-group of 4 heads).
Each core computes a partial output  ctx_g @ Wo[g*256:(g+1)*256, :]  for its
batch; the host sums the 4 partials per batch and adds the bias.

Perf notes (measured on hw, baseline 490us -> 287us):
- The indexer path (x^T, qi/ki projections, idx scores) stays true fp32: with
  fp32r the hw top-64 selection drifts from the fp32 reference at score-gap
  boundaries (42 rows > 1e-2, rel err 2.8e-2 > the 2e-2 gate).
- q/k/v/out-proj matmuls run fp32r (1 cycle/row); probs and v are fp16.
- The top-k additive mask is accumulated into the score PSUM with a
  bf16-identity matmul instead of a DVE tensor add.
- softmax probs are normalized on the Act engine (Copy activation with a
  per-partition reciprocal scale). gpsimd/Pool TensorScalarPtr is a Q7
  software op at ~6.8us per call and single-handedly cost the old kernel
  ~220us -- never put per-element work on gpsimd here.
- prob transposes are fp16 PE transposes (1 cycle/row) into fp16 PSUM, copied
  by DVE (2x_1p mode). DmaTransposeAnt is NOT usable for blocked 3D outputs:
  walrus lowers it differently from CoreSim's semantic model (it even stomps
  unrelated SBUF), and its fixed +16 DMA semaphores break Tile's wait
  accounting when >1 transpose feeds one consumer.
- Weight loads are one batched DMA each, straight into fp32r tiles (bitcast
  DRAM APs); x is loaded in 4 chunked DMAs and transposed per token-half so
  the indexer projections + chunk-0..3 top-k (serial DVE chain, the critical
  resource) start while the second half of x is still in flight.
- Emission interleaves idx chunks 4..7 with the first token-group's attention
  heads, with each chunk's 8 top-k rounds SPLIT across two heads so the
  attention chain's short DVE ops (recips, probT copies) wait behind at most
  ~4 serial top-k rounds in the in-order DVE queue. Weight DMAs are emitted
  after the first x-chunk DMA (x feeds the critical-path transposes).
  Tried and measured WORSE than this arrangement: two unsplit idx chunks
  after the first head (305us), all four idx chunks after all of tg0's heads
  (299us AND wrong results -- a latent sync bug surfaces under that order),
  normalize on DVE instead of Act (296us), normalize fused into the transpose
  as a diag(1/den) matmul (289us), xTr copies on Act (301us).
"""

import numpy as np

import concourse.bacc as bacc
import concourse.bass as bass
import concourse.mybir as mybir
import concourse.tile as tile
from concourse import masks
from concourse.bass_utils import run_bass_kernel_spmd

F32 = mybir.dt.float32
F32R = mybir.dt.float32r
F16 = mybir.dt.float16
BF16 = mybir.dt.bfloat16
AF = mybir.ActivationFunctionType
ALU = mybir.AluOpType

B, T, D = 2, 1024, 1024
H, HD = 16, 64          # total heads, head dim
HG = 4                  # heads per core
HI, IHD = 4, 64         # index heads, index head dim
TOPK = 64
NCHUNK = T // 128       # 8 token chunks of 128
NEG = -3.0e30           # causal-invalid marker (additive mask value)
SENT = -1.0e30          # match_replace sentinel (distinct from NEG)

_NEFF_CACHE = "/var/tmp/bass-neff-cache"


def _install_neff_cache():
    """walrus compile output cache keyed on BIR hash (compiles are minutes)."""
    import hashlib
    import os
    import shutil

    import concourse.bass2jax as b2j

    if getattr(b2j, "_dsa_neff_cache_installed", False):
        return
    orig = b2j.compile_bir_kernel

    def cached(bir_json, tmpdir, neff_name="file.neff"):
        try:
            h = hashlib.sha256(
                bir_json if isinstance(bir_json, bytes) else bir_json.encode()
            ).hexdigest()[:24]
            os.makedirs(_NEFF_CACHE, exist_ok=True)
            hit = os.path.join(_NEFF_CACHE, h + ".neff")
            if os.path.exists(hit):
                dst = os.path.join(tmpdir, neff_name)
                shutil.copyfile(hit, dst)
                return dst
            neff = orig(bir_json, tmpdir, neff_name)
            shutil.copyfile(neff, hit + ".tmp")
            os.replace(hit + ".tmp", hit)
            return neff
        except OSError:
            return orig(bir_json, tmpdir, neff_name)

    b2j.compile_bir_kernel = cached
    b2j._dsa_neff_cache_installed = True


def R(ap):
    return ap.bitcast(F32R)


def build_kernel(tc, out_ap, x_ap, wq_ap, wk_ap, wv_ap, wo_ap, wi_ap, dbg=None):
    """Emit the per-core kernel. All APs are DRAM tensors:
    x [1024,1024], wq/wk/wv [1024,256], wo [256,1024],
    wi [1024,324] = concat(Wqi[1024,256], Wki[1024,64], Ww[1024,4]).
    out [1024,1024] partial (pre-bias, pre-reduction over head groups).
    """
    nc = tc.nc
    import os
    from contextlib import ExitStack
    skip_attn = os.environ.get("K_SKIP_ATTN") == "1"
    skip_topk = os.environ.get("K_SKIP_TOPK") == "1"
    attn_no_sm = os.environ.get("K_ATTN_NO_SM") == "1"   # scores+mask only
    attn_no_tp = os.environ.get("K_ATTN_NO_TP") == "1"   # + exp/recip/normalize
    stack = ExitStack()

    const_pool = stack.enter_context(tc.tile_pool(name="const", bufs=1))
    ident_b = const_pool.tile([128, 128], BF16)
    masks.make_identity(nc, ident_b[:])
    ident_h = const_pool.tile([128, 128], F16)
    masks.make_identity(nc, ident_h[:])
    ident_f = const_pool.tile([128, 128], F32)
    masks.make_identity(nc, ident_f[:])
    causal = const_pool.tile([128, 128], F32)
    masks.make_causal_mask(nc, causal[:], mask_val=NEG)

    # ---- weights: one DMA each, straight into f32r tiles ----
    w_pool = stack.enter_context(tc.tile_pool(name="weights", bufs=1))
    wq_sb = w_pool.tile([128, 8 * 256], F32R)
    wk_sb = w_pool.tile([128, 8 * 256], F32R)
    wv_sb = w_pool.tile([128, 8 * 256], F32R)
    wo_sb = w_pool.tile([128, 2 * 1024], F32R)
    wi_sb = w_pool.tile([128, 8 * 324], F32)

    def emit_weight_dmas():
        # emitted after the first x-chunk DMA: x feeds the critical-path
        # transposes, the weights have slack until B1/B2
        nc.sync.dma_start(
            wi_sb[:].rearrange("p (j c) -> p j c", c=324),
            wi_ap.rearrange("(j p) c -> p j c", p=128))
        for ap_, dst_, c in ((wq_ap, wq_sb, 256), (wk_ap, wk_sb, 256),
                             (wv_ap, wv_sb, 256), (wo_ap, wo_sb, 1024)):
            nc.sync.dma_start(
                dst_[:].rearrange("p (j c) -> p j c", c=c),
                R(ap_.rearrange("(j p) c -> p j c", p=128)))

    act_pool = stack.enter_context(tc.tile_pool(name="acts", bufs=1))
    qT = act_pool.tile([128, 2 * 1024], F32R)    # heads (2m,2m+1) rows, tokens free
    kT = act_pool.tile([128, 2 * 1024], F32R)
    qiT = act_pool.tile([128, 2 * 1024], F32)
    kiw = act_pool.tile([128, 1024], F32)        # rows 0-63 kiT, 64-67 wT logits
    kiw2 = act_pool.tile([128, 1024], F32)       # rows 64-127: copy of kiT (odd index heads)
    v_sb = act_pool.tile([128, 8 * 256], F16)    # [s-chunk sc] at cols sc*256, head cols inside
    ctxT = act_pool.tile([128, 2 * 1024], F32R)  # [ck] at cols ck*1024
    w8 = act_pool.tile([128, 32], F32)           # softmax(x@Ww)/8, chunk i at cols 4i
    mask_tiles = [act_pool.tile([128, (i + 1) * 128], BF16, name=f"mask{i}",
                                tag=f"mask{i}") for i in range(NCHUNK)]

    idx_pool = stack.enter_context(tc.tile_pool(name="idx", bufs=1))

    big_ps = stack.enter_context(tc.tile_pool(name="big_ps", bufs=2, space="PSUM"))
    small_ps = stack.enter_context(tc.tile_pool(name="small_ps", bufs=2, space="PSUM"))
    tp16_ps = stack.enter_context(tc.tile_pool(name="tp16_ps", bufs=2, space="PSUM"))

    idx_state = {}

    def emit_idx_head(i, head_rounds):
        n_s = (i + 1) * 128
        if skip_topk:
            return
        work = idx_pool.tile([128, 1024], F32, name="work", tag="work", bufs=4)
        for h in range(HI):
            m, r = h // 2, (h % 2) * 64
            ps = big_ps.tile([128, 1024], F32, name="ips", tag="mm")
            for grp in range((n_s + 511) // 512):
                ns0, ns1 = grp * 512, min(n_s, (grp + 1) * 512)
                ki_rhs = kiw[0:64, ns0:ns1] if r == 0 else kiw2[64:128, ns0:ns1]
                nc.tensor.matmul(
                    ps[:, ns0:ns1],
                    qiT[r:r + 64, m * 1024 + i * 128: m * 1024 + (i + 1) * 128],
                    ki_rhs, start=True, stop=True)
            if h == 0:
                nc.scalar.activation(work[:, 0:n_s], ps[:, 0:n_s], AF.Relu,
                                     scale=w8[:, i * 4 + h: i * 4 + h + 1])
            else:
                aw = idx_pool.tile([128, 1024], F32, name="aw", tag="aw", bufs=3)
                nc.scalar.activation(aw[:, 0:n_s], ps[:, 0:n_s], AF.Relu,
                                     scale=w8[:, i * 4 + h: i * 4 + h + 1])
                nc.gpsimd.tensor_tensor(work[:, 0:n_s], work[:, 0:n_s],
                                        aw[:, 0:n_s], op=ALU.add)
        nc.gpsimd.tensor_tensor(work[:, i * 128:(i + 1) * 128],
                                work[:, i * 128:(i + 1) * 128], causal[:], op=ALU.add)
        tmax = idx_pool.tile([128, 8], F32, name="tmax", tag="tmax", bufs=2)
        for _ in range(head_rounds):
            nc.vector.max(tmax[:], work[:, 0:n_s])
            nc.vector.match_replace(work[:, 0:n_s], tmax[:], work[:, 0:n_s], SENT)
        idx_state[i] = (work, tmax, n_s, head_rounds)

    def emit_idx_tail(i):
        if skip_topk:
            mk = mask_tiles[i]
            nc.gpsimd.memset(mk[:], 0.0)
            nc.gpsimd.tensor_tensor(mk[:, i * 128:(i + 1) * 128],
                                    mk[:, i * 128:(i + 1) * 128], causal[:], op=ALU.add)
            return
        work, tmax, n_s, head_rounds = idx_state.pop(i)
        for _ in range(8 - head_rounds):
            nc.vector.max(tmax[:], work[:, 0:n_s])
            nc.vector.match_replace(work[:, 0:n_s], tmax[:], work[:, 0:n_s], SENT)
        mk = mask_tiles[i]
        nc.vector.tensor_scalar(mk[:], work[:, 0:n_s], SENT, NEG,
                                op0=ALU.not_equal, op1=ALU.mult)
        nc.gpsimd.tensor_tensor(mk[:, i * 128:(i + 1) * 128],
                                mk[:, i * 128:(i + 1) * 128], causal[:], op=ALU.add)

    def emit_idx(i):
        emit_idx_head(i, 8)
        emit_idx_tail(i)

    with tc.tile_pool(name="xscope", bufs=1) as xscope, \
         tc.tile_pool(name="xtok", bufs=2) as xtok_pool:
        xT = xscope.tile([128, 8 * 1024], F32)   # [d-chunk j] at cols j*1024, feature-major
        xTr = xscope.tile([128, 8 * 1024], F32R)  # rounded shadow for f32r matmuls
        xT3 = xT[:].rearrange("p (j c) -> p j c", c=1024)
        xTr3 = xTr[:].rearrange("p (j c) -> p j c", c=1024)

        def emit_b1_half(tg):
            """Indexer projections for token-half tg (needs only xT token cols
            tg*512..(tg+1)*512, i.e. token chunks tg*4..tg*4+3 transposed)."""
            for m in range(2):  # qiT
                ps = small_ps.tile([128, 512], F32, name="b1", tag="sps")
                for j in range(8):
                    nc.tensor.matmul(
                        ps[:],
                        wi_sb[:, j * 324 + m * 128: j * 324 + (m + 1) * 128],
                        xT[:, j * 1024 + tg * 512: j * 1024 + (tg + 1) * 512],
                        start=(j == 0), stop=(j == 7))
                nc.scalar.copy(qiT[:, m * 1024 + tg * 512: m * 1024 + (tg + 1) * 512],
                               ps[:])
            ps = small_ps.tile([128, 512], F32, name="b1k", tag="sps")
            for j in range(8):  # kiT + wT logits (68 cols of wi)
                nc.tensor.matmul(
                    ps[0:68, :],
                    wi_sb[:, j * 324 + 256: j * 324 + 324],
                    xT[:, j * 1024 + tg * 512: j * 1024 + (tg + 1) * 512],
                    start=(j == 0), stop=(j == 7))
            nc.scalar.copy(kiw[0:68, tg * 512:(tg + 1) * 512], ps[0:68, :])
            nc.sync.dma_start(kiw2[64:128, tg * 512:(tg + 1) * 512],
                              kiw[0:64, tg * 512:(tg + 1) * 512])
            for i in range(tg * 4, tg * 4 + 4):
                # w softmax per chunk: transpose wT logits [4, t128] -> [t128, 4]
                pw = small_ps.tile([128, 512], F32, name="pw", tag="sps")
                nc.tensor.transpose(pw[:, 0:4], kiw[64:68, i * 128:(i + 1) * 128],
                                    ident_f[64:68, 64:68])
                wexp = act_pool.tile([128, 4], F32, name="wexp", tag="wexp", bufs=2)
                wden = act_pool.tile([128, 1], F32, name="wden", tag="wden", bufs=2)
                nc.scalar.activation(wexp[:], pw[:, 0:4], AF.Exp, accum_out=wden[:])
                wrec = act_pool.tile([128, 1], F32, name="wrec", tag="wrec", bufs=2)
                nc.vector.reciprocal(wrec[:], wden[:])
                nc.vector.tensor_scalar(w8[:, i * 4:(i + 1) * 4], wexp[:], wrec[:],
                                        0.125, op0=ALU.mult, op1=ALU.mult)

        # ---- Phase A + B1, pipelined by token-half ----
        for half in range(2):
            for hq in range(2):
                xt = xtok_pool.tile([128, 2 * 1024], F32, name="xt", tag="xtok")
                nc.sync.dma_start(
                    xt[:].rearrange("p (i c) -> p i c", c=1024),
                    x_ap.rearrange("(i p) c -> p i c", p=128)[
                        :, half * 4 + hq * 2: half * 4 + (hq + 1) * 2, :])
                if half == 0 and hq == 0:
                    emit_weight_dmas()
                for q in range(2):
                    i = half * 4 + hq * 2 + q
                    pt = big_ps.tile([128, 1024], F32, name="pt", tag="mm")
                    for j in range(8):
                        nc.tensor.matmul(
                            pt[:, j * 128:(j + 1) * 128],
                            xt[:, q * 1024 + j * 128: q * 1024 + (j + 1) * 128],
                            ident_f[:], is_transpose=True, start=True, stop=True)
                    nc.scalar.copy(xT3[:, :, i * 128:(i + 1) * 128],
                                   pt[:].rearrange("p (j c) -> p j c", c=128))
            # gpsimd-issued casting DMA: keeps the f32r shadow copy off the
            # DVE queue (where it would serialize behind the top-k runs and
            # gate B2 -> attention start)
            nc.gpsimd.dma_start(xTr3[:, :, half * 512:(half + 1) * 512],
                                xT3[:, :, half * 512:(half + 1) * 512])
            emit_b1_half(half)
            if half == 0:
                for i in range(4):
                    emit_idx(i)

        # ---- Phase B2: q/k/v projections ----
        for (wsb, dst_) in ((wq_sb, qT), (wk_sb, kT)):
            for m in range(2):
                ps = big_ps.tile([128, 1024], F32, name="qps", tag="mm")
                for tg in range(2):
                    for j in range(8):
                        nc.tensor.matmul(
                            ps[:, tg * 512:(tg + 1) * 512],
                            wsb[:, j * 256 + m * 128: j * 256 + (m + 1) * 128],
                            xTr[:, j * 1024 + tg * 512: j * 1024 + (tg + 1) * 512],
                            start=(j == 0), stop=(j == 7))
                nc.scalar.copy(dst_[:, m * 1024:(m + 1) * 1024], ps[:])
        for sc in range(8):  # v natural layout: out [s128, 256] per s-chunk
            ps = small_ps.tile([128, 512], F32, name="vps", tag="sps")
            for j in range(8):
                nc.tensor.matmul(
                    ps[:, 0:256],
                    xTr[:, j * 1024 + sc * 128: j * 1024 + (sc + 1) * 128],
                    wv_sb[:, j * 256:(j + 1) * 256],
                    start=(j == 0), stop=(j == 7))
            nc.scalar.copy(v_sb[:, sc * 256:(sc + 1) * 256], ps[:, 0:256])

    # ---- Phases C+D: attention, with idx(4..7) interleaved ----
    with tc.tile_pool(name="attn", bufs=1) as attn_pool, \
         tc.tile_pool(name="attn2", bufs=1) as attn2_pool:

        def emit_attn_head(tg, h):
            m, r = h // 2, (h % 2) * 64
            probT = attn_pool.tile([128, 8, 1024], F16, name="probT", tag="probT",
                                   bufs=2)
            for i in range(tg * 4, tg * 4 + 4):
                n_s = (i + 1) * 128
                ps = big_ps.tile([128, 1024], F32, name="aps", tag="mm")
                for grp in range((n_s + 511) // 512):
                    ns0, ns1 = grp * 512, min(n_s, (grp + 1) * 512)
                    nc.tensor.matmul(
                        ps[:, ns0:ns1],
                        qT[r:r + 64, m * 1024 + i * 128: m * 1024 + (i + 1) * 128],
                        kT[r:r + 64, m * 1024 + ns0: m * 1024 + ns1],
                        start=True, stop=False)
                    nc.tensor.matmul(ps[:, ns0:ns1], ident_b[:],
                                     mask_tiles[i][:, ns0:ns1],
                                     start=False, stop=True)
                if attn_no_sm:
                    continue
                scr = attn2_pool.tile([128, 1024], F16, name="scr", tag="scr", bufs=4)
                den = attn2_pool.tile([128, 1], F32, name="den", tag="den", bufs=4)
                nc.scalar.activation(scr[:, 0:n_s], ps[:, 0:n_s], AF.Exp,
                                     scale=0.125, accum_out=den[:])
                rec = attn2_pool.tile([128, 1], F32, name="rec", tag="rec", bufs=4)
                nc.vector.reciprocal(rec[:], den[:])
                prob = attn2_pool.tile([128, 1024], F16, name="prob", tag="prob",
                                       bufs=4)
                nc.scalar.activation(prob[:, 0:n_s], scr[:, 0:n_s], AF.Copy,
                                     scale=rec[:])
                if attn_no_tp:
                    continue
                tp = tp16_ps.tile([128, 1024], F16, name="tp", tag="tp16")
                for sc in range(i + 1):
                    nc.tensor.matmul(tp[:, sc * 128:(sc + 1) * 128],
                                     prob[:, sc * 128:(sc + 1) * 128], ident_h[:],
                                     is_transpose=True, start=True, stop=True)
                nc.vector.tensor_copy(
                    probT[:, 0:i + 1, i * 128:(i + 1) * 128],
                    tp[:, 0:n_s].rearrange("p (a b) -> p a b", b=128))
            if attn_no_sm or attn_no_tp:
                return
            pc = small_ps.tile([128, 512], F32, name="pc", tag="sps")
            n_sc = tg * 4 + 4
            for sc in range(n_sc):
                off = max(sc - tg * 4, 0) * 128
                nc.tensor.matmul(
                    pc[0:64, off:512],
                    v_sb[:, sc * 256 + h * 64: sc * 256 + (h + 1) * 64],
                    probT[:, sc, tg * 512 + off:(tg + 1) * 512],
                    start=(sc == 0), stop=(sc == n_sc - 1))
            ck, rr = h // 2, (h % 2) * 64
            nc.scalar.copy(ctxT[rr:rr + 64, ck * 1024 + tg * 512: ck * 1024 + (tg + 1) * 512],
                           pc[0:64, :])

        def emit_outproj_i(i, dve_copy=False):
            ps = big_ps.tile([128, 1024], F32, name="ops", tag="mm")
            for og in range(2):
                for ck in range(2):
                    nc.tensor.matmul(
                        ps[:, og * 512:(og + 1) * 512],
                        ctxT[:, ck * 1024 + i * 128: ck * 1024 + (i + 1) * 128],
                        wo_sb[:, ck * 1024 + og * 512: ck * 1024 + (og + 1) * 512],
                        start=(ck == 0), stop=(ck == 1))
            out_sb = attn2_pool.tile([128, 1024], F32, name="out_sb", tag="out",
                                     bufs=2)
            if dve_copy:
                nc.vector.tensor_copy(out_sb[:], ps[:])
            else:
                nc.scalar.copy(out_sb[:], ps[:])
            nc.sync.dma_start(out_ap[i * 128:(i + 1) * 128, :], out_sb[:])

        def emit_outproj(tg, dve_copy=False):
            for i in range(tg * 4, tg * 4 + 4):
                emit_outproj_i(i, dve_copy)

        if skip_attn or attn_no_sm or attn_no_tp:
            if not skip_attn:
                for h in range(HG):
                    emit_attn_head(0, h)
                    emit_idx(4 + h)
                for h in range(HG):
                    emit_attn_head(1, h)
            else:
                for h in range(HG):
                    emit_idx(4 + h)
            for i in range(NCHUNK):
                out_sb = attn2_pool.tile([128, 1024], F32, name="out_sb", tag="out",
                                         bufs=2)
                nc.vector.tensor_copy(out_sb[:], qT[:].bitcast(F32)[:, 0:1024])
                nc.sync.dma_start(out_ap[i * 128:(i + 1) * 128, :], out_sb[:])
        else:
            # topk(4..7) split into 4-round emission pieces so each attention
            # head's short DVE ops wait behind at most half a chunk's serial
            # top-k run in the in-order DVE queue
            emit_attn_head(0, 0)
            emit_idx_head(4, 4)
            emit_attn_head(0, 1)
            emit_idx_tail(4)
            emit_idx_head(5, 4)
            emit_attn_head(0, 2)
            emit_idx_tail(5)
            emit_idx_head(6, 4)
            emit_attn_head(0, 3)
            emit_idx_tail(6)
            emit_idx(7)
            # outproj(0) interleaved per token-chunk after each tg1 head: its
            # Act copies/PE matmuls stay hidden inside the tg1 window instead
            # of delaying tg1's first exp/scores in the in-order queues
            for h in range(HG):
                emit_attn_head(1, h)
                emit_outproj_i(h)
            emit_outproj(1)

        if dbg is not None:
            def dump(name, ap):
                if name in dbg:
                    nc.sync.dma_start(dbg[name], ap.bitcast(dbg[name].dtype))
            dump("qiT", qiT[:])
            dump("kiw", kiw[0:68, :])
            dump("kiw2", kiw2[64:128, :])
            dump("w8", w8[:])
            dump("qT", qT[:])
            dump("kT", kT[:])
            dump("v", v_sb[:])
            dump("ctxT", ctxT[:])
            for i in range(NCHUNK):
                dump(f"mask{i}", mask_tiles[i][:])

    stack.close()


def _build_nc(loop=0):
    nc = bacc.Bacc("TRN2")
    x = nc.dram_tensor("x", [T, D], F32, kind="ExternalInput")
    wq = nc.dram_tensor("wq", [D, 256], F32, kind="ExternalInput")
    wk = nc.dram_tensor("wk", [D, 256], F32, kind="ExternalInput")
    wv = nc.dram_tensor("wv", [D, 256], F32, kind="ExternalInput")
    wo = nc.dram_tensor("wo", [256, D], F32, kind="ExternalInput")
    wi = nc.dram_tensor("wi", [D, 324], F32, kind="ExternalInput")
    out = nc.dram_tensor("out", [T, D], F32, kind="ExternalOutput")
    with tile.TileContext(nc) as tc:
        if loop:
            with tc.For_i(0, loop, 1):
                build_kernel(tc, out.ap(), x.ap(), wq.ap(), wk.ap(), wv.ap(), wo.ap(), wi.ap())
        else:
            build_kernel(tc, out.ap(), x.ap(), wq.ap(), wk.ap(), wv.ap(), wo.ap(), wi.ap())
    nc.compile()
    return nc


def kernel(x, Wq, Wk, Wv, Wo, bo, Wqi, Wki, Ww, _trace=False):
    _install_neff_cache()
    x, Wq, Wk, Wv, Wo, bo, Wqi, Wki, Ww = (
        np.ascontiguousarray(np.asarray(a, np.float32))
        for a in (x, Wq, Wk, Wv, Wo, bo, Wqi, Wki, Ww))
    nc = _build_nc()
    in_maps = _make_in_maps(x, Wq, Wk, Wv, Wo, Wqi, Wki, Ww)
    res = run_bass_kernel_spmd(nc, in_maps, core_ids=list(range(8)), trace=_trace)
    outs = [r["out"] for r in res.results]
    full = np.stack([sum(outs[b * 4:(b + 1) * 4]) + bo for b in range(B)], axis=0)
    full = full.astype(np.float32)
    if _trace:
        return full, res
    return full


def _make_in_maps(x, Wq, Wk, Wv, Wo, Wqi, Wki, Ww):
    wi = np.ascontiguousarray(np.concatenate([Wqi, Wki, Ww], axis=1))
    in_maps = []
    for b in range(B):
        for g in range(4):
            c = slice(g * 256, (g + 1) * 256)
            in_maps.append({
                "x": np.ascontiguousarray(x[b]),
                "wq": np.ascontiguousarray(Wq[:, c]),
                "wk": np.ascontiguousarray(Wk[:, c]),
                "wv": np.ascontiguousarray(Wv[:, c]),
                "wo": np.ascontiguousarray(Wo[c, :]),
                "wi": wi,
            })
    return in_maps


def bench_exec_ns(inputs, iters=10, loop=256):
    """Per-iteration device time: the kernel body loops `loop` times inside one
    NEFF; dispatch-overhead floor (loop=1 variant) is subtracted via the slope
    between two loop counts. Returns ns per kernel iteration."""
    lo = max(1, loop // 8)
    t_hi = _bench_exec_wall(inputs, iters, loop)
    t_lo = _bench_exec_wall(inputs, iters, lo)
    return (t_hi - t_lo) / (loop - lo) * 1e9


def _bench_exec_wall(inputs, iters, loop):
    import time

    import jax
    from jax.experimental.shard_map import shard_map
    from jax.sharding import Mesh, NamedSharding, PartitionSpec

    import concourse.bass2jax as b2j

    _install_neff_cache()
    b2j.install_neuronx_cc_hook()
    nc = _build_nc(loop=loop)
    ins = {k: np.ascontiguousarray(np.asarray(v, np.float32)) for k, v in inputs.items()}
    in_maps = _make_in_maps(ins["x"], ins["Wq"], ins["Wk"], ins["Wv"], ins["Wo"],
                            ins["Wqi"], ins["Wki"], ins["Ww"])

    partition_name = nc.partition_id_tensor.name if nc.partition_id_tensor else None
    in_names, out_names, out_avals, zero_outs = [], [], [], []
    for alloc in nc.m.functions[0].allocations:
        if not isinstance(alloc, mybir.MemoryLocationSet):
            continue
        name = alloc.memorylocations[0].name
        if alloc.kind == "ExternalInput":
            if name != partition_name:
                in_names.append(name)
        elif alloc.kind == "ExternalOutput":
            shape = tuple(alloc.tensor_shape)
            dtype = mybir.dt.np(alloc.dtype)
            out_names.append(name)
            out_avals.append(jax.core.ShapedArray(shape, dtype))
            zero_outs.append(np.zeros(shape, dtype))
    n_params = len(in_names)
    all_in_names = list(in_names) + list(out_names)
    if partition_name is not None:
        all_in_names.append(partition_name)

    def _body(*args):
        operands = list(args)
        if partition_name is not None:
            operands.append(b2j.partition_id_tensor())
        outs = b2j._bass_exec_p.bind(
            *operands,
            out_avals=tuple(out_avals),
            in_names=tuple(all_in_names),
            out_names=tuple(out_names),
            lowering_input_output_aliases=(),
            sim_require_finite=True,
            sim_require_nnan=True,
            nc=nc,
        )
        return tuple(outs)

    n_cores = len(in_maps)
    devices = jax.devices()[:n_cores]
    mesh = Mesh(np.asarray(devices), ("core",))
    in_specs = (PartitionSpec("core"),) * (n_params + len(out_names))
    out_specs = (PartitionSpec("core"),) * len(out_names)
    fn = jax.jit(shard_map(_body, mesh=mesh, in_specs=in_specs,
                           out_specs=out_specs, check_rep=False))
    sharding = NamedSharding(mesh, PartitionSpec("core"))
    dev_args = [
        jax.device_put(
            np.concatenate([np.asarray(in_maps[c][nm]) for c in range(n_cores)], axis=0),
            sharding)
        for nm in in_names
    ] + [
        jax.device_put(np.concatenate([z] * n_cores, axis=0), sharding)
        for z in zero_outs
    ]
    r = fn(*dev_args)
    jax.block_until_ready(r)
    times = []
    for _ in range(iters):
        t0 = time.perf_counter()
        r = fn(*dev_args)
        jax.block_until_ready(r)
        times.append(time.perf_counter() - t0)
    return min(times)


if __name__ == "__main__":
    rng = np.random.default_rng(0)
    ins = {
        "x": rng.standard_normal((B, T, D)).astype(np.float32),
        "Wq": (rng.standard_normal((D, D)) * 0.02).astype(np.float32),
        "Wk": (rng.standard_normal((D, D)) * 0.02).astype(np.float32),
        "Wv": (rng.standard_normal((D, D)) * 0.02).astype(np.float32),
        "Wo": (rng.standard_normal((D, D)) * 0.02).astype(np.float32),
        "bo": np.zeros(D, np.float32),
        "Wqi": (rng.standard_normal((D, HI * IHD)) * 0.02).astype(np.float32),
        "Wki": (rng.standard_normal((D, IHD)) * 0.02).astype(np.float32),
        "Ww": (rng.standard_normal((D, HI)) * 0.02).astype(np.float32),
    }
    out = kernel(**ins)
    print("out", out.shape, out.dtype, float(np.abs(out).max()))


# revision 63
# speedup vs baseline: 1.0165x; 1.0165x over previous
"""Bass/Trainium2 kernel for MultiHeadAttentionWithDSA (sparse attention with
lightning-indexer top-64 key selection), sharded over 8 NeuronCores.

Sharding: core = b*4 + g  (b in {0,1} batch, g in {0..3} head-group of 4 heads).
Each core computes a partial output  ctx_g @ Wo[g*256:(g+1)*256, :]  for its
batch; the host sums the 4 partials per batch and adds the bias.

Perf notes (measured on hw, baseline 490us -> 287us):
- The indexer path (x^T, qi/ki projections, idx scores) stays true fp32: with
  fp32r the hw top-64 selection drifts from the fp32 reference at score-gap
  boundaries (42 rows > 1e-2, rel err 2.8e-2 > the 2e-2 gate).
- q/k/v/out-proj matmuls run fp32r (1 cycle/row); probs and v are fp16.
- The top-k additive mask is accumulated into the score PSUM with a
  bf16-identity matmul instead of a DVE tensor add.
- softmax probs are normalized on the Act engine (Copy activation with a
  per-partition reciprocal scale). gpsimd/Pool TensorScalarPtr is a Q7
  software op at ~6.8us per call and single-handedly cost the old kernel
  ~220us -- never put per-element work on gpsimd here.
- prob transposes are fp16 PE transposes (1 cycle/row) into fp16 PSUM, copied
  by DVE (2x_1p mode). DmaTransposeAnt is NOT usable for blocked 3D outputs:
  walrus lowers it differently from CoreSim's semantic model (it even stomps
  unrelated SBUF), and its fixed +16 DMA semaphores break Tile's wait
  accounting when >1 transpose feeds one consumer.
- Weight loads are one batched DMA each, straight into fp32r tiles (bitcast
  DRAM APs); x is loaded in 4 chunked DMAs and transposed per token-half so
  the indexer projections + chunk-0..3 top-k (serial DVE chain, the critical
  resource) start while the second half of x is still in flight.
- Emission interleaves idx chunks 4..7 with the first token-group's attention
  heads, with each chunk's 8 top-k rounds SPLIT across two heads so the
  attention chain's short DVE ops (recips, probT copies) wait behind at most
  ~4 serial top-k rounds in the in-order DVE queue. Weight DMAs are emitted
  after the first x-chunk DMA (x feeds the critical-path transposes).
  Tried and measured WORSE than this arrangement: two unsplit idx chunks
  after the first head (305us), all four idx chunks after all of tg0's heads
  (299us AND wrong results -- a latent sync bug surfaces under that order),
  normalize on DVE instead of Act (296us), normalize fused into the transpose
  as a diag(1/den) matmul (289us), xTr copies on Act (301us).
"""

import numpy as np

import concourse.bacc as bacc
import concourse.bass as bass
import concourse.mybir as mybir
import concourse.tile as tile
from concourse import masks
from concourse.bass_utils import run_bass_kernel_spmd

F32 = mybir.dt.float32
F32R = mybir.dt.float32r
F16 = mybir.dt.float16
BF16 = mybir.dt.bfloat16
AF = mybir.ActivationFunctionType
ALU = mybir.AluOpType

B, T, D = 2, 1024, 1024
H, HD = 16, 64          # total heads, head dim
HG = 4                  # heads per core
HI, IHD = 4, 64         # index heads, index head dim
TOPK = 64
NCHUNK = T // 128       # 8 token chunks of 128
NEG = -3.0e30           # causal-invalid marker (additive mask value)
SENT = -1.0e30          # match_replace sentinel (distinct from NEG)

_NEFF_CACHE = "/var/tmp/bass-neff-cache"


def _install_neff_cache():
    """walrus compile output cache keyed on BIR hash (compiles are minutes)."""
    import hashlib
    import os
    import shutil

    import concourse.bass2jax as b2j

    if getattr(b2j, "_dsa_neff_cache_installed", False):
        return
    orig = b2j.compile_bir_kernel

    def cached(bir_json, tmpdir, neff_name="file.neff"):
        try:
            h = hashlib.sha256(
                bir_json if isinstance(bir_json, bytes) else bir_json.encode()
            ).hexdigest()[:24]
            os.makedirs(_NEFF_CACHE, exist_ok=True)
            hit = os.path.join(_NEFF_CACHE, h + ".neff")
            if os.path.exists(hit):
                dst = os.path.join(tmpdir, neff_name)
                shutil.copyfile(hit, dst)
                return dst
            neff = orig(bir_json, tmpdir, neff_name)
            shutil.copyfile(neff, hit + ".tmp")
            os.replace(hit + ".tmp", hit)
            return neff
        except OSError:
            return orig(bir_json, tmpdir, neff_name)

    b2j.compile_bir_kernel = cached
    b2j._dsa_neff_cache_installed = True


def R(ap):
    return ap.bitcast(F32R)


def build_kernel(tc, out_ap, x_ap, wq_ap, wk_ap, wv_ap, wo_ap, wi_ap, dbg=None):
    """Emit the per-core kernel. All APs are DRAM tensors:
    x [1024,1024], wq/wk/wv [1024,256], wo [256,1024],
    wi [1024,324] = concat(Wqi[1024,256], Wki[1024,64], Ww[1024,4]).
    out [1024,1024] partial (pre-bias, pre-reduction over head groups).
    """
    nc = tc.nc
    import os
    from contextlib import ExitStack
    skip_attn = os.environ.get("K_SKIP_ATTN") == "1"
    skip_topk = os.environ.get("K_SKIP_TOPK") == "1"
    attn_no_sm = os.environ.get("K_ATTN_NO_SM") == "1"   # scores+mask only
    attn_no_tp = os.environ.get("K_ATTN_NO_TP") == "1"   # + exp/recip/normalize
    stack = ExitStack()

    const_pool = stack.enter_context(tc.tile_pool(name="const", bufs=1))
    ident_b = const_pool.tile([128, 128], BF16)
    masks.make_identity(nc, ident_b[:])
    ident_h = const_pool.tile([128, 128], F16)
    masks.make_identity(nc, ident_h[:])
    ident_f = const_pool.tile([128, 128], F32)
    masks.make_identity(nc, ident_f[:])
    causal = const_pool.tile([128, 128], F32)
    masks.make_causal_mask(nc, causal[:], mask_val=NEG)

    # ---- weights: one DMA each, straight into f32r tiles ----
    w_pool = stack.enter_context(tc.tile_pool(name="weights", bufs=1))
    wq_sb = w_pool.tile([128, 8 * 256], F32R)
    wk_sb = w_pool.tile([128, 8 * 256], F32R)
    wv_sb = w_pool.tile([128, 8 * 256], F32R)
    wo_sb = w_pool.tile([128, 2 * 1024], F32R)
    wi_sb = w_pool.tile([128, 8 * 324], F32)

    def emit_weight_dmas():
        # emitted after the first x-chunk DMA: x feeds the critical-path
        # transposes, the weights have slack until B1/B2
        nc.sync.dma_start(
            wi_sb[:].rearrange("p (j c) -> p j c", c=324),
            wi_ap.rearrange("(j p) c -> p j c", p=128))
        for ap_, dst_, c in ((wq_ap, wq_sb, 256), (wk_ap, wk_sb, 256),
                             (wv_ap, wv_sb, 256), (wo_ap, wo_sb, 1024)):
            nc.sync.dma_start(
                dst_[:].rearrange("p (j c) -> p j c", c=c),
                R(ap_.rearrange("(j p) c -> p j c", p=128)))

    act_pool = stack.enter_context(tc.tile_pool(name="acts", bufs=1))
    qT = act_pool.tile([128, 2 * 1024], F32R)    # heads (2m,2m+1) rows, tokens free
    kT = act_pool.tile([128, 2 * 1024], F32R)
    qiT = act_pool.tile([128, 2 * 1024], F32)
    kiw = act_pool.tile([128, 1024], F32)        # rows 0-63 kiT, 64-67 wT logits
    kiw2 = act_pool.tile([128, 1024], F32)       # rows 64-127: copy of kiT (odd index heads)
    v_sb = act_pool.tile([128, 8 * 256], F16)    # [s-chunk sc] at cols sc*256, head cols inside
    ctxT = act_pool.tile([128, 2 * 1024], F32R)  # [ck] at cols ck*1024
    w8 = act_pool.tile([128, 32], F32)           # softmax(x@Ww)/8, chunk i at cols 4i
    mask_tiles = [act_pool.tile([128, (i + 1) * 128], BF16, name=f"mask{i}",
                                tag=f"mask{i}") for i in range(NCHUNK)]

    idx_pool = stack.enter_context(tc.tile_pool(name="idx", bufs=1))

    big_ps = stack.enter_context(tc.tile_pool(name="big_ps", bufs=2, space="PSUM"))
    small_ps = stack.enter_context(tc.tile_pool(name="small_ps", bufs=2, space="PSUM"))
    tp16_ps = stack.enter_context(tc.tile_pool(name="tp16_ps", bufs=2, space="PSUM"))

    idx_state = {}

    def emit_idx_head(i, head_rounds):
        n_s = (i + 1) * 128
        if skip_topk:
            return
        work = idx_pool.tile([128, 1024], F32, name="work", tag="work", bufs=4)
        for h in range(HI):
            m, r = h // 2, (h % 2) * 64
            ps = big_ps.tile([128, 1024], F32, name="ips", tag="mm")
            for grp in range((n_s + 511) // 512):
                ns0, ns1 = grp * 512, min(n_s, (grp + 1) * 512)
                ki_rhs = kiw[0:64, ns0:ns1] if r == 0 else kiw2[64:128, ns0:ns1]
                nc.tensor.matmul(
                    ps[:, ns0:ns1],
                    qiT[r:r + 64, m * 1024 + i * 128: m * 1024 + (i + 1) * 128],
                    ki_rhs, start=True, stop=True)
            if h == 0:
                nc.scalar.activation(work[:, 0:n_s], ps[:, 0:n_s], AF.Relu,
                                     scale=w8[:, i * 4 + h: i * 4 + h + 1])
            else:
                aw = idx_pool.tile([128, 1024], F32, name="aw", tag="aw", bufs=3)
                nc.scalar.activation(aw[:, 0:n_s], ps[:, 0:n_s], AF.Relu,
                                     scale=w8[:, i * 4 + h: i * 4 + h + 1])
                nc.gpsimd.tensor_tensor(work[:, 0:n_s], work[:, 0:n_s],
                                        aw[:, 0:n_s], op=ALU.add)
        nc.gpsimd.tensor_tensor(work[:, i * 128:(i + 1) * 128],
                                work[:, i * 128:(i + 1) * 128], causal[:], op=ALU.add)
        tmax = idx_pool.tile([128, 8], F32, name="tmax", tag="tmax", bufs=2)
        for _ in range(head_rounds):
            nc.vector.max(tmax[:], work[:, 0:n_s])
            nc.vector.match_replace(work[:, 0:n_s], tmax[:], work[:, 0:n_s], SENT)
        idx_state[i] = (work, tmax, n_s, head_rounds)

    def emit_idx_tail(i):
        if skip_topk:
            mk = mask_tiles[i]
            nc.gpsimd.memset(mk[:], 0.0)
            nc.gpsimd.tensor_tensor(mk[:, i * 128:(i + 1) * 128],
                                    mk[:, i * 128:(i + 1) * 128], causal[:], op=ALU.add)
            return
        work, tmax, n_s, head_rounds = idx_state.pop(i)
        for _ in range(8 - head_rounds):
            nc.vector.max(tmax[:], work[:, 0:n_s])
            nc.vector.match_replace(work[:, 0:n_s], tmax[:], work[:, 0:n_s], SENT)
        mk = mask_tiles[i]
        nc.vector.tensor_scalar(mk[:], work[:, 0:n_s], SENT, NEG,
                                op0=ALU.not_equal, op1=ALU.mult)
        nc.gpsimd.tensor_tensor(mk[:, i * 128:(i + 1) * 128],
                                mk[:, i * 128:(i + 1) * 128], causal[:], op=ALU.add)

    def emit_idx(i):
        emit_idx_head(i, 8)
        emit_idx_tail(i)

    with tc.tile_pool(name="xscope", bufs=1) as xscope, \
         tc.tile_pool(name="xtok", bufs=2) as xtok_pool:
        xT = xscope.tile([128, 8 * 1024], F32)   # [d-chunk j] at cols j*1024, feature-major
        xTr = xscope.tile([128, 8 * 1024], F32R)  # rounded shadow for f32r matmuls
        xT3 = xT[:].rearrange("p (j c) -> p j c", c=1024)
        xTr3 = xTr[:].rearrange("p (j c) -> p j c", c=1024)

        def emit_b1_half(tg):
            """Indexer projections for token-half tg (needs only xT token cols
            tg*512..(tg+1)*512, i.e. token chunks tg*4..tg*4+3 transposed)."""
            for m in range(2):  # qiT
                ps = small_ps.tile([128, 512], F32, name="b1", tag="sps")
                for j in range(8):
                    nc.tensor.matmul(
                        ps[:],
                        wi_sb[:, j * 324 + m * 128: j * 324 + (m + 1) * 128],
                        xT[:, j * 1024 + tg * 512: j * 1024 + (tg + 1) * 512],
                        start=(j == 0), stop=(j == 7))
                nc.scalar.copy(qiT[:, m * 1024 + tg * 512: m * 1024 + (tg + 1) * 512],
                               ps[:])
            ps = small_ps.tile([128, 512], F32, name="b1k", tag="sps")
            for j in range(8):  # kiT + wT logits (68 cols of wi)
                nc.tensor.matmul(
                    ps[0:68, :],
                    wi_sb[:, j * 324 + 256: j * 324 + 324],
                    xT[:, j * 1024 + tg * 512: j * 1024 + (tg + 1) * 512],
                    start=(j == 0), stop=(j == 7))
            nc.scalar.copy(kiw[0:68, tg * 512:(tg + 1) * 512], ps[0:68, :])
            nc.sync.dma_start(kiw2[64:128, tg * 512:(tg + 1) * 512],
                              kiw[0:64, tg * 512:(tg + 1) * 512])
            for i in range(tg * 4, tg * 4 + 4):
                # w softmax per chunk: transpose wT logits [4, t128] -> [t128, 4]
                pw = small_ps.tile([128, 512], F32, name="pw", tag="sps")
                nc.tensor.transpose(pw[:, 0:4], kiw[64:68, i * 128:(i + 1) * 128],
                                    ident_f[64:68, 64:68])
                wexp = act_pool.tile([128, 4], F32, name="wexp", tag="wexp", bufs=2)
                wden = act_pool.tile([128, 1], F32, name="wden", tag="wden", bufs=2)
                nc.scalar.activation(wexp[:], pw[:, 0:4], AF.Exp, accum_out=wden[:])
                wrec = act_pool.tile([128, 1], F32, name="wrec", tag="wrec", bufs=2)
                nc.vector.reciprocal(wrec[:], wden[:])
                nc.vector.tensor_scalar(w8[:, i * 4:(i + 1) * 4], wexp[:], wrec[:],
                                        0.125, op0=ALU.mult, op1=ALU.mult)

        # ---- Phase A + B1, pipelined by token-half ----
        for half in range(2):
            for hq in range(2):
                xt = xtok_pool.tile([128, 2 * 1024], F32, name="xt", tag="xtok")
                nc.sync.dma_start(
                    xt[:].rearrange("p (i c) -> p i c", c=1024),
                    x_ap.rearrange("(i p) c -> p i c", p=128)[
                        :, half * 4 + hq * 2: half * 4 + (hq + 1) * 2, :])
                if half == 0 and hq == 0:
                    emit_weight_dmas()
                for q in range(2):
                    i = half * 4 + hq * 2 + q
                    pt = big_ps.tile([128, 1024], F32, name="pt", tag="mm")
                    for j in range(8):
                        nc.tensor.matmul(
                            pt[:, j * 128:(j + 1) * 128],
                            xt[:, q * 1024 + j * 128: q * 1024 + (j + 1) * 128],
                            ident_f[:], is_transpose=True, start=True, stop=True)
                    nc.scalar.copy(xT3[:, :, i * 128:(i + 1) * 128],
                                   pt[:].rearrange("p (j c) -> p j c", c=128))
            # gpsimd-issued casting DMA: keeps the f32r shadow copy off the
            # DVE queue (where it would serialize behind the top-k runs and
            # gate B2 -> attention start)
            nc.gpsimd.dma_start(xTr3[:, :, half * 512:(half + 1) * 512],
                                xT3[:, :, half * 512:(half + 1) * 512])
            emit_b1_half(half)
            if half == 0:
                for i in range(4):
                    emit_idx(i)

        # ---- Phase B2: q/k/v projections ----
        for (wsb, dst_) in ((wq_sb, qT), (wk_sb, kT)):
            for m in range(2):
                ps = big_ps.tile([128, 1024], F32, name="qps", tag="mm")
                for tg in range(2):
                    for j in range(8):
                        nc.tensor.matmul(
                            ps[:, tg * 512:(tg + 1) * 512],
                            wsb[:, j * 256 + m * 128: j * 256 + (m + 1) * 128],
                            xTr[:, j * 1024 + tg * 512: j * 1024 + (tg + 1) * 512],
                            start=(j == 0), stop=(j == 7))
                nc.scalar.copy(dst_[:, m * 1024:(m + 1) * 1024], ps[:])
        for sc in range(8):  # v natural layout: out [s128, 256] per s-chunk
            ps = small_ps.tile([128, 512], F32, name="vps", tag="sps")
            for j in range(8):
                nc.tensor.matmul(
                    ps[:, 0:256],
                    xTr[:, j * 1024 + sc * 128: j * 1024 + (sc + 1) * 128],
                    wv_sb[:, j * 256:(j + 1) * 256],
                    start=(j == 0), stop=(j == 7))
            nc.scalar.copy(v_sb[:, sc * 256:(sc + 1) * 256], ps[:, 0:256])

    # ---- Phases C+D: attention, with idx(4..7) interleaved ----
    with tc.tile_pool(name="attn", bufs=1) as attn_pool, \
         tc.tile_pool(name="attn2", bufs=1) as attn2_pool:

        def emit_attn_head(tg, h):
            m, r = h // 2, (h % 2) * 64
            probT = attn_pool.tile([128, 8, 1024], F16, name="probT", tag="probT",
                                   bufs=2)
            for i in range(tg * 4, tg * 4 + 4):
                n_s = (i + 1) * 128
                ps = big_ps.tile([128, 1024], F32, name="aps", tag="mm")
                for grp in range((n_s + 511) // 512):
                    ns0, ns1 = grp * 512, min(n_s, (grp + 1) * 512)
                    nc.tensor.matmul(
                        ps[:, ns0:ns1],
                        qT[r:r + 64, m * 1024 + i * 128: m * 1024 + (i + 1) * 128],
                        kT[r:r + 64, m * 1024 + ns0: m * 1024 + ns1],
                        start=True, stop=False)
                    nc.tensor.matmul(ps[:, ns0:ns1], ident_b[:],
                                     mask_tiles[i][:, ns0:ns1],
                                     start=False, stop=True)
                if attn_no_sm:
                    continue
                scr = attn2_pool.tile([128, 1024], F16, name="scr", tag="scr", bufs=4)
                den = attn2_pool.tile([128, 1], F32, name="den", tag="den", bufs=4)
                nc.scalar.activation(scr[:, 0:n_s], ps[:, 0:n_s], AF.Exp,
                                     scale=0.125, accum_out=den[:])
                rec = attn2_pool.tile([128, 1], F32, name="rec", tag="rec", bufs=4)
                nc.vector.reciprocal(rec[:], den[:])
                prob = attn2_pool.tile([128, 1024], F16, name="prob", tag="prob",
                                       bufs=4)
                nc.scalar.activation(prob[:, 0:n_s], scr[:, 0:n_s], AF.Copy,
                                     scale=rec[:])
                if attn_no_tp:
                    continue
                tp = tp16_ps.tile([128, 1024], F16, name="tp", tag="tp16")
                for sc in range(i + 1):
                    nc.tensor.matmul(tp[:, sc * 128:(sc + 1) * 128],
                                     prob[:, sc * 128:(sc + 1) * 128], ident_h[:],
                                     is_transpose=True, start=True, stop=True)
                nc.vector.tensor_copy(
                    probT[:, 0:i + 1, i * 128:(i + 1) * 128],
                    tp[:, 0:n_s].rearrange("p (a b) -> p a b", b=128))
            if attn_no_sm or attn_no_tp:
                return
            pc = small_ps.tile([128, 512], F32, name="pc", tag="sps")
            n_sc = tg * 4 + 4
            for sc in range(n_sc):
                off = max(sc - tg * 4, 0) * 128
                nc.tensor.matmul(
                    pc[0:64, off:512],
                    v_sb[:, sc * 256 + h * 64: sc * 256 + (h + 1) * 64],
                    probT[:, sc, tg * 512 + off:(tg + 1) * 512],
                    start=(sc == 0), stop=(sc == n_sc - 1))
            ck, rr = h // 2, (h % 2) * 64
            nc.scalar.copy(ctxT[rr:rr + 64, ck * 1024 + tg * 512: ck * 1024 + (tg + 1) * 512],
                           pc[0:64, :])

        def emit_outproj(tg, dve_copy=False):
            for i in range(tg * 4, tg * 4 + 4):
                ps = big_ps.tile([128, 1024], F32, name="ops", tag="mm")
                for og in range(2):
                    for ck in range(2):
                        nc.tensor.matmul(
                            ps[:, og * 512:(og + 1) * 512],
                            ctxT[:, ck * 1024 + i * 128: ck * 1024 + (i + 1) * 128],
                            wo_sb[:, ck * 1024 + og * 512: ck * 1024 + (og + 1) * 512],
                            start=(ck == 0), stop=(ck == 1))
                out_sb = attn2_pool.tile([128, 1024], F32, name="out_sb", tag="out",
                                         bufs=2)
                if dve_copy:
                    nc.vector.tensor_copy(out_sb[:], ps[:])
                else:
                    nc.scalar.copy(out_sb[:], ps[:])
                nc.sync.dma_start(out_ap[i * 128:(i + 1) * 128, :], out_sb[:])

        if skip_attn or attn_no_sm or attn_no_tp:
            if not skip_attn:
                for h in range(HG):
                    emit_attn_head(0, h)
                    emit_idx(4 + h)
                for h in range(HG):
                    emit_attn_head(1, h)
            else:
                for h in range(HG):
                    emit_idx(4 + h)
            for i in range(NCHUNK):
                out_sb = attn2_pool.tile([128, 1024], F32, name="out_sb", tag="out",
                                         bufs=2)
                nc.vector.tensor_copy(out_sb[:], qT[:].bitcast(F32)[:, 0:1024])
                nc.sync.dma_start(out_ap[i * 128:(i + 1) * 128, :], out_sb[:])
        else:
            # topk(4..7) split into 4-round emission pieces so each attention
            # head's short DVE ops wait behind at most half a chunk's serial
            # top-k run in the in-order DVE queue
            emit_attn_head(0, 0)
            emit_idx_head(4, 4)
            emit_attn_head(0, 1)
            emit_idx_tail(4)
            emit_idx_head(5, 4)
            emit_attn_head(0, 2)
            emit_idx_tail(5)
            emit_idx_head(6, 4)
            emit_attn_head(0, 3)
            emit_idx_tail(6)
            emit_idx(7)
            emit_outproj(0)
            for h in range(HG):
                emit_attn_head(1, h)
            emit_outproj(1)

        if dbg is not None:
            def dump(name, ap):
                if name in dbg:
                    nc.sync.dma_start(dbg[name], ap.bitcast(dbg[name].dtype))
            dump("qiT", qiT[:])
            dump("kiw", kiw[0:68, :])
            dump("kiw2", kiw2[64:128, :])
            dump("w8", w8[:])
            dump("qT", qT[:])
            dump("kT", kT[:])
            dump("v", v_sb[:])
            dump("ctxT", ctxT[:])
            for i in range(NCHUNK):
                dump(f"mask{i}", mask_tiles[i][:])

    stack.close()


def _build_nc(loop=0):
    nc = bacc.Bacc("TRN2")
    x = nc.dram_tensor("x", [T, D], F32, kind="ExternalInput")
    wq = nc.dram_tensor("wq", [D, 256], F32, kind="ExternalInput")
    wk = nc.dram_tensor("wk", [D, 256], F32, kind="ExternalInput")
    wv = nc.dram_tensor("wv", [D, 256], F32, kind="ExternalInput")
    wo = nc.dram_tensor("wo", [256, D], F32, kind="ExternalInput")
    wi = nc.dram_tensor("wi", [D, 324], F32, kind="ExternalInput")
    out = nc.dram_tensor("out", [T, D], F32, kind="ExternalOutput")
    with tile.TileContext(nc) as tc:
        if loop:
            with tc.For_i(0, loop, 1):
                build_kernel(tc, out.ap(), x.ap(), wq.ap(), wk.ap(), wv.ap(), wo.ap(), wi.ap())
        else:
            build_kernel(tc, out.ap(), x.ap(), wq.ap(), wk.ap(), wv.ap(), wo.ap(), wi.ap())
    nc.compile()
    return nc


def kernel(x, Wq, Wk, Wv, Wo, bo, Wqi, Wki, Ww, _trace=False):
    _install_neff_cache()
    x, Wq, Wk, Wv, Wo, bo, Wqi, Wki, Ww = (
        np.ascontiguousarray(np.asarray(a, np.float32))
        for a in (x, Wq, Wk, Wv, Wo, bo, Wqi, Wki, Ww))
    nc = _build_nc()
    in_maps = _make_in_maps(x, Wq, Wk, Wv, Wo, Wqi, Wki, Ww)
    res = run_bass_kernel_spmd(nc, in_maps, core_ids=list(range(8)), trace=_trace)
    outs = [r["out"] for r in res.results]
    full = np.stack([sum(outs[b * 4:(b + 1) * 4]) + bo for b in range(B)], axis=0)
    full = full.astype(np.float32)
    if _trace:
        return full, res
    return full


def _make_in_maps(x, Wq, Wk, Wv, Wo, Wqi, Wki, Ww):
    wi = np.ascontiguousarray(np.concatenate([Wqi, Wki, Ww], axis=1))
    in_maps = []
    for b in range(B):
        for g in range(4):
            c = slice(g * 256, (g + 1) * 256)
            in_maps.append({
                "x": np.ascontiguousarray(x[b]),
                "wq": np.ascontiguousarray(Wq[:, c]),
                "wk": np.ascontiguousarray(Wk[:, c]),
                "wv": np.ascontiguousarray(Wv[:, c]),
                "wo": np.ascontiguousarray(Wo[c, :]),
                "wi": wi,
            })
    return in_maps


def bench_exec_ns(inputs, iters=10, loop=256):
    """Per-iteration device time: the kernel body loops `loop` times inside one
    NEFF; dispatch-overhead floor (loop=1 variant) is subtracted via the slope
    between two loop counts. Returns ns per kernel iteration."""
    lo = max(1, loop // 8)
    t_hi = _bench_exec_wall(inputs, iters, loop)
    t_lo = _bench_exec_wall(inputs, iters, lo)
    return (t_hi - t_lo) / (loop - lo) * 1e9


def _bench_exec_wall(inputs, iters, loop):
    import time

    import jax
    from jax.experimental.shard_map import shard_map
    from jax.sharding import Mesh, NamedSharding, PartitionSpec

    import concourse.bass2jax as b2j

    _install_neff_cache()
    b2j.install_neuronx_cc_hook()
    nc = _build_nc(loop=loop)
    ins = {k: np.ascontiguousarray(np.asarray(v, np.float32)) for k, v in inputs.items()}
    in_maps = _make_in_maps(ins["x"], ins["Wq"], ins["Wk"], ins["Wv"], ins["Wo"],
                            ins["Wqi"], ins["Wki"], ins["Ww"])

    partition_name = nc.partition_id_tensor.name if nc.partition_id_tensor else None
    in_names, out_names, out_avals, zero_outs = [], [], [], []
    for alloc in nc.m.functions[0].allocations:
        if not isinstance(alloc, mybir.MemoryLocationSet):
            continue
        name = alloc.memorylocations[0].name
        if alloc.kind == "ExternalInput":
            if name != partition_name:
                in_names.append(name)
        elif alloc.kind == "ExternalOutput":
            shape = tuple(alloc.tensor_shape)
            dtype = mybir.dt.np(alloc.dtype)
            out_names.append(name)
            out_avals.append(jax.core.ShapedArray(shape, dtype))
            zero_outs.append(np.zeros(shape, dtype))
    n_params = len(in_names)
    all_in_names = list(in_names) + list(out_names)
    if partition_name is not None:
        all_in_names.append(partition_name)

    def _body(*args):
        operands = list(args)
        if partition_name is not None:
            operands.append(b2j.partition_id_tensor())
        outs = b2j._bass_exec_p.bind(
            *operands,
            out_avals=tuple(out_avals),
            in_names=tuple(all_in_names),
            out_names=tuple(out_names),
            lowering_input_output_aliases=(),
            sim_require_finite=True,
            sim_require_nnan=True,
            nc=nc,
        )
        return tuple(outs)

    n_cores = len(in_maps)
    devices = jax.devices()[:n_cores]
    mesh = Mesh(np.asarray(devices), ("core",))
    in_specs = (PartitionSpec("core"),) * (n_params + len(out_names))
    out_specs = (PartitionSpec("core"),) * len(out_names)
    fn = jax.jit(shard_map(_body, mesh=mesh, in_specs=in_specs,
                           out_specs=out_specs, check_rep=False))
    sharding = NamedSharding(mesh, PartitionSpec("core"))
    dev_args = [
        jax.device_put(
            np.concatenate([np.asarray(in_maps[c][nm]) for c in range(n_cores)], axis=0),
            sharding)
        for nm in in_names
    ] + [
        jax.device_put(np.concatenate([z] * n_cores, axis=0), sharding)
        for z in zero_outs
    ]
    r = fn(*dev_args)
    jax.block_until_ready(r)
    times = []
    for _ in range(iters):
        t0 = time.perf_counter()
        r = fn(*dev_args)
        jax.block_until_ready(r)
        times.append(time.perf_counter() - t0)
    return min(times)


if __name__ == "__main__":
    rng = np.random.default_rng(0)
    ins = {
        "x": rng.standard_normal((B, T, D)).astype(np.float32),
        "Wq": (rng.standard_normal((D, D)) * 0.02).astype(np.float32),
        "Wk": (rng.standard_normal((D, D)) * 0.02).astype(np.float32),
        "Wv": (rng.standard_normal((D, D)) * 0.02).astype(np.float32),
        "Wo": (rng.standard_normal((D, D)) * 0.02).astype(np.float32),
        "bo": np.zeros(D, np.float32),
        "Wqi": (rng.standard_normal((D, HI * IHD)) * 0.02).astype(np.float32),
        "Wki": (rng.standard_normal((D, IHD)) * 0.02).astype(np.float32),
        "Ww": (rng.standard_normal((D, HI)) * 0.02).astype(np.float32),
    }
    out = kernel(**ins)
    print("out", out.shape, out.dtype, float(np.abs(out).max()))


# revision 64
# speedup vs baseline: 1.0433x; 1.0264x over previous
"""Bass/Trainium2 kernel for MultiHeadAttentionWithDSA (sparse attention with
lightning-indexer top-64 key selection), sharded over 8 NeuronCores.

Sharding: core = b*4 + g  (b in {0,1} batch, g in {0..3} head-group of 4 heads).
Each core computes a partial output  ctx_g @ Wo[g*256:(g+1)*256, :]  for its
batch; the host sums the 4 partials per batch and adds the bias.

Perf notes (measured on hw, baseline 490us -> 287us):
- The indexer path (x^T, qi/ki projections, idx scores) stays true fp32: with
  fp32r the hw top-64 selection drifts from the fp32 reference at score-gap
  boundaries (42 rows > 1e-2, rel err 2.8e-2 > the 2e-2 gate).
- q/k/v/out-proj matmuls run fp32r (1 cycle/row); probs and v are fp16.
- The top-k additive mask is accumulated into the score PSUM with a
  bf16-identity matmul instead of a DVE tensor add.
- softmax probs are normalized on the Act engine (Copy activation with a
  per-partition reciprocal scale). gpsimd/Pool TensorScalarPtr is a Q7
  software op at ~6.8us per call and single-handedly cost the old kernel
  ~220us -- never put per-element work on gpsimd here.
- prob transposes are fp16 PE transposes (1 cycle/row) into fp16 PSUM, copied
  by DVE (2x_1p mode). DmaTransposeAnt is NOT usable for blocked 3D outputs:
  walrus lowers it differently from CoreSim's semantic model (it even stomps
  unrelated SBUF), and its fixed +16 DMA semaphores break Tile's wait
  accounting when >1 transpose feeds one consumer.
- Weight loads are one batched DMA each, straight into fp32r tiles (bitcast
  DRAM APs); x is loaded in 4 chunked DMAs and transposed per token-half so
  the indexer projections + chunk-0..3 top-k (serial DVE chain, the critical
  resource) start while the second half of x is still in flight.
- Emission interleaves idx chunks 4..7 with the first token-group's attention
  heads, with each chunk's 8 top-k rounds SPLIT across two heads so the
  attention chain's short DVE ops (recips, probT copies) wait behind at most
  ~4 serial top-k rounds in the in-order DVE queue. Weight DMAs are emitted
  after the first x-chunk DMA (x feeds the critical-path transposes).
  Tried and measured WORSE than this arrangement: two unsplit idx chunks
  after the first head (305us), all four idx chunks after all of tg0's heads
  (299us AND wrong results -- a latent sync bug surfaces under that order),
  normalize on DVE instead of Act (296us), normalize fused into the transpose
  as a diag(1/den) matmul (289us), xTr copies on Act (301us).
"""

import numpy as np

import concourse.bacc as bacc
import concourse.bass as bass
import concourse.mybir as mybir
import concourse.tile as tile
from concourse import masks
from concourse.bass_utils import run_bass_kernel_spmd

F32 = mybir.dt.float32
F32R = mybir.dt.float32r
F16 = mybir.dt.float16
BF16 = mybir.dt.bfloat16
AF = mybir.ActivationFunctionType
ALU = mybir.AluOpType

B, T, D = 2, 1024, 1024
H, HD = 16, 64          # total heads, head dim
HG = 4                  # heads per core
HI, IHD = 4, 64         # index heads, index head dim
TOPK = 64
NCHUNK = T // 128       # 8 token chunks of 128
NEG = -3.0e30           # causal-invalid marker (additive mask value)
SENT = -1.0e30          # match_replace sentinel (distinct from NEG)

_NEFF_CACHE = "/var/tmp/bass-neff-cache"


def _install_neff_cache():
    """walrus compile output cache keyed on BIR hash (compiles are minutes)."""
    import hashlib
    import os
    import shutil

    import concourse.bass2jax as b2j

    if getattr(b2j, "_dsa_neff_cache_installed", False):
        return
    orig = b2j.compile_bir_kernel

    def cached(bir_json, tmpdir, neff_name="file.neff"):
        try:
            h = hashlib.sha256(
                bir_json if isinstance(bir_json, bytes) else bir_json.encode()
            ).hexdigest()[:24]
            os.makedirs(_NEFF_CACHE, exist_ok=True)
            hit = os.path.join(_NEFF_CACHE, h + ".neff")
            if os.path.exists(hit):
                dst = os.path.join(tmpdir, neff_name)
                shutil.copyfile(hit, dst)
                return dst
            neff = orig(bir_json, tmpdir, neff_name)
            shutil.copyfile(neff, hit + ".tmp")
            os.replace(hit + ".tmp", hit)
            return neff
        except OSError:
            return orig(bir_json, tmpdir, neff_name)

    b2j.compile_bir_kernel = cached
    b2j._dsa_neff_cache_installed = True


def R(ap):
    return ap.bitcast(F32R)


def build_kernel(tc, out_ap, x_ap, wq_ap, wk_ap, wv_ap, wo_ap, wi_ap, dbg=None):
    """Emit the per-core kernel. All APs are DRAM tensors:
    x [1024,1024], wq/wk/wv [1024,256], wo [256,1024],
    wi [1024,324] = concat(Wqi[1024,256], Wki[1024,64], Ww[1024,4]).
    out [1024,1024] partial (pre-bias, pre-reduction over head groups).
    """
    nc = tc.nc
    import os
    from contextlib import ExitStack
    skip_attn = os.environ.get("K_SKIP_ATTN") == "1"
    skip_topk = os.environ.get("K_SKIP_TOPK") == "1"
    attn_no_sm = os.environ.get("K_ATTN_NO_SM") == "1"   # scores+mask only
    attn_no_tp = os.environ.get("K_ATTN_NO_TP") == "1"   # + exp/recip/normalize
    stack = ExitStack()

    const_pool = stack.enter_context(tc.tile_pool(name="const", bufs=1))
    ident_b = const_pool.tile([128, 128], BF16)
    masks.make_identity(nc, ident_b[:])
    ident_h = const_pool.tile([128, 128], F16)
    masks.make_identity(nc, ident_h[:])
    ident_f = const_pool.tile([128, 128], F32)
    masks.make_identity(nc, ident_f[:])
    causal = const_pool.tile([128, 128], F32)
    masks.make_causal_mask(nc, causal[:], mask_val=NEG)

    # ---- weights: one DMA each, straight into f32r tiles ----
    w_pool = stack.enter_context(tc.tile_pool(name="weights", bufs=1))
    wq_sb = w_pool.tile([128, 8 * 256], F32R)
    wk_sb = w_pool.tile([128, 8 * 256], F32R)
    wv_sb = w_pool.tile([128, 8 * 256], F32R)
    wo_sb = w_pool.tile([128, 2 * 1024], F32R)
    wi_sb = w_pool.tile([128, 8 * 324], F32)

    def emit_wi_dma():
        # wi feeds B1 (~15us in); the x chunk DMAs ahead of it feed the
        # critical-path transposes
        nc.sync.dma_start(
            wi_sb[:].rearrange("p (j c) -> p j c", c=324),
            wi_ap.rearrange("(j p) c -> p j c", p=128))

    def emit_qkvo_dmas():
        # wq/wk/wv have slack until B2, wo until outproj: emitted after ALL
        # x-chunk DMAs so they never delay the transpose pipeline
        for ap_, dst_, c in ((wq_ap, wq_sb, 256), (wk_ap, wk_sb, 256),
                             (wv_ap, wv_sb, 256), (wo_ap, wo_sb, 1024)):
            nc.sync.dma_start(
                dst_[:].rearrange("p (j c) -> p j c", c=c),
                R(ap_.rearrange("(j p) c -> p j c", p=128)))

    act_pool = stack.enter_context(tc.tile_pool(name="acts", bufs=1))
    qT = act_pool.tile([128, 2 * 1024], F32R)    # heads (2m,2m+1) rows, tokens free
    kT = act_pool.tile([128, 2 * 1024], F32R)
    qiT = act_pool.tile([128, 2 * 1024], F32)
    kiw = act_pool.tile([128, 1024], F32)        # rows 0-63 kiT, 64-67 wT logits
    kiw2 = act_pool.tile([128, 1024], F32)       # rows 64-127: copy of kiT (odd index heads)
    v_sb = act_pool.tile([128, 8 * 256], F16)    # [s-chunk sc] at cols sc*256, head cols inside
    ctxT = act_pool.tile([128, 2 * 1024], F32R)  # [ck] at cols ck*1024
    w8 = act_pool.tile([128, 32], F32)           # softmax(x@Ww)/8, chunk i at cols 4i
    mask_tiles = [act_pool.tile([128, (i + 1) * 128], BF16, name=f"mask{i}",
                                tag=f"mask{i}") for i in range(NCHUNK)]

    idx_pool = stack.enter_context(tc.tile_pool(name="idx", bufs=1))

    big_ps = stack.enter_context(tc.tile_pool(name="big_ps", bufs=2, space="PSUM"))
    small_ps = stack.enter_context(tc.tile_pool(name="small_ps", bufs=2, space="PSUM"))
    tp16_ps = stack.enter_context(tc.tile_pool(name="tp16_ps", bufs=2, space="PSUM"))

    idx_state = {}

    def emit_idx_head(i, head_rounds):
        n_s = (i + 1) * 128
        if skip_topk:
            return
        work = idx_pool.tile([128, 1024], F32, name="work", tag="work", bufs=4)
        for h in range(HI):
            m, r = h // 2, (h % 2) * 64
            ps = big_ps.tile([128, 1024], F32, name="ips", tag="mm")
            for grp in range((n_s + 511) // 512):
                ns0, ns1 = grp * 512, min(n_s, (grp + 1) * 512)
                ki_rhs = kiw[0:64, ns0:ns1] if r == 0 else kiw2[64:128, ns0:ns1]
                nc.tensor.matmul(
                    ps[:, ns0:ns1],
                    qiT[r:r + 64, m * 1024 + i * 128: m * 1024 + (i + 1) * 128],
                    ki_rhs, start=True, stop=True)
            if h == 0:
                nc.scalar.activation(work[:, 0:n_s], ps[:, 0:n_s], AF.Relu,
                                     scale=w8[:, i * 4 + h: i * 4 + h + 1])
            else:
                aw = idx_pool.tile([128, 1024], F32, name="aw", tag="aw", bufs=3)
                nc.scalar.activation(aw[:, 0:n_s], ps[:, 0:n_s], AF.Relu,
                                     scale=w8[:, i * 4 + h: i * 4 + h + 1])
                nc.gpsimd.tensor_tensor(work[:, 0:n_s], work[:, 0:n_s],
                                        aw[:, 0:n_s], op=ALU.add)
        nc.gpsimd.tensor_tensor(work[:, i * 128:(i + 1) * 128],
                                work[:, i * 128:(i + 1) * 128], causal[:], op=ALU.add)
        tmax = idx_pool.tile([128, 8], F32, name="tmax", tag="tmax", bufs=2)
        for _ in range(head_rounds):
            nc.vector.max(tmax[:], work[:, 0:n_s])
            nc.vector.match_replace(work[:, 0:n_s], tmax[:], work[:, 0:n_s], SENT)
        idx_state[i] = (work, tmax, n_s, head_rounds)

    def emit_idx_tail(i):
        if skip_topk:
            mk = mask_tiles[i]
            nc.gpsimd.memset(mk[:], 0.0)
            nc.gpsimd.tensor_tensor(mk[:, i * 128:(i + 1) * 128],
                                    mk[:, i * 128:(i + 1) * 128], causal[:], op=ALU.add)
            return
        work, tmax, n_s, head_rounds = idx_state.pop(i)
        for _ in range(8 - head_rounds):
            nc.vector.max(tmax[:], work[:, 0:n_s])
            nc.vector.match_replace(work[:, 0:n_s], tmax[:], work[:, 0:n_s], SENT)
        mk = mask_tiles[i]
        nc.vector.tensor_scalar(mk[:], work[:, 0:n_s], SENT, NEG,
                                op0=ALU.not_equal, op1=ALU.mult)
        nc.gpsimd.tensor_tensor(mk[:, i * 128:(i + 1) * 128],
                                mk[:, i * 128:(i + 1) * 128], causal[:], op=ALU.add)

    def emit_idx(i):
        emit_idx_head(i, 8)
        emit_idx_tail(i)

    with tc.tile_pool(name="xscope", bufs=1) as xscope, \
         tc.tile_pool(name="xtok", bufs=2) as xtok_pool:
        xT = xscope.tile([128, 8 * 1024], F32)   # [d-chunk j] at cols j*1024, feature-major
        xTr = xscope.tile([128, 8 * 1024], F32R)  # rounded shadow for f32r matmuls
        xT3 = xT[:].rearrange("p (j c) -> p j c", c=1024)
        xTr3 = xTr[:].rearrange("p (j c) -> p j c", c=1024)

        def emit_b1_half(tg):
            """Indexer projections for token-half tg (needs only xT token cols
            tg*512..(tg+1)*512, i.e. token chunks tg*4..tg*4+3 transposed)."""
            for m in range(2):  # qiT
                ps = small_ps.tile([128, 512], F32, name="b1", tag="sps")
                for j in range(8):
                    nc.tensor.matmul(
                        ps[:],
                        wi_sb[:, j * 324 + m * 128: j * 324 + (m + 1) * 128],
                        xT[:, j * 1024 + tg * 512: j * 1024 + (tg + 1) * 512],
                        start=(j == 0), stop=(j == 7))
                nc.scalar.copy(qiT[:, m * 1024 + tg * 512: m * 1024 + (tg + 1) * 512],
                               ps[:])
            ps = small_ps.tile([128, 512], F32, name="b1k", tag="sps")
            for j in range(8):  # kiT + wT logits (68 cols of wi)
                nc.tensor.matmul(
                    ps[0:68, :],
                    wi_sb[:, j * 324 + 256: j * 324 + 324],
                    xT[:, j * 1024 + tg * 512: j * 1024 + (tg + 1) * 512],
                    start=(j == 0), stop=(j == 7))
            nc.scalar.copy(kiw[0:68, tg * 512:(tg + 1) * 512], ps[0:68, :])
            nc.sync.dma_start(kiw2[64:128, tg * 512:(tg + 1) * 512],
                              kiw[0:64, tg * 512:(tg + 1) * 512])
            for i in range(tg * 4, tg * 4 + 4):
                # w softmax per chunk: transpose wT logits [4, t128] -> [t128, 4]
                pw = small_ps.tile([128, 512], F32, name="pw", tag="sps")
                nc.tensor.transpose(pw[:, 0:4], kiw[64:68, i * 128:(i + 1) * 128],
                                    ident_f[64:68, 64:68])
                wexp = act_pool.tile([128, 4], F32, name="wexp", tag="wexp", bufs=2)
                wden = act_pool.tile([128, 1], F32, name="wden", tag="wden", bufs=2)
                nc.scalar.activation(wexp[:], pw[:, 0:4], AF.Exp, accum_out=wden[:])
                wrec = act_pool.tile([128, 1], F32, name="wrec", tag="wrec", bufs=2)
                nc.vector.reciprocal(wrec[:], wden[:])
                nc.vector.tensor_scalar(w8[:, i * 4:(i + 1) * 4], wexp[:], wrec[:],
                                        0.125, op0=ALU.mult, op1=ALU.mult)

        # ---- Phase A + B1, pipelined by token-half ----
        for half in range(2):
            for hq in range(2):
                xt = xtok_pool.tile([128, 2 * 1024], F32, name="xt", tag="xtok")
                nc.sync.dma_start(
                    xt[:].rearrange("p (i c) -> p i c", c=1024),
                    x_ap.rearrange("(i p) c -> p i c", p=128)[
                        :, half * 4 + hq * 2: half * 4 + (hq + 1) * 2, :])
                if half == 0 and hq == 1:
                    emit_wi_dma()
                if half == 1 and hq == 1:
                    emit_qkvo_dmas()
                for q in range(2):
                    i = half * 4 + hq * 2 + q
                    pt = big_ps.tile([128, 1024], F32, name="pt", tag="mm")
                    for j in range(8):
                        nc.tensor.matmul(
                            pt[:, j * 128:(j + 1) * 128],
                            xt[:, q * 1024 + j * 128: q * 1024 + (j + 1) * 128],
                            ident_f[:], is_transpose=True, start=True, stop=True)
                    nc.scalar.copy(xT3[:, :, i * 128:(i + 1) * 128],
                                   pt[:].rearrange("p (j c) -> p j c", c=128))
            # gpsimd-issued casting DMA: keeps the f32r shadow copy off the
            # DVE queue (where it would serialize behind the top-k runs and
            # gate B2 -> attention start)
            nc.gpsimd.dma_start(xTr3[:, :, half * 512:(half + 1) * 512],
                                xT3[:, :, half * 512:(half + 1) * 512])
            emit_b1_half(half)
            if half == 0:
                for i in range(4):
                    emit_idx(i)

        # ---- Phase B2: q/k/v projections ----
        for (wsb, dst_) in ((wq_sb, qT), (wk_sb, kT)):
            for m in range(2):
                ps = big_ps.tile([128, 1024], F32, name="qps", tag="mm")
                for tg in range(2):
                    for j in range(8):
                        nc.tensor.matmul(
                            ps[:, tg * 512:(tg + 1) * 512],
                            wsb[:, j * 256 + m * 128: j * 256 + (m + 1) * 128],
                            xTr[:, j * 1024 + tg * 512: j * 1024 + (tg + 1) * 512],
                            start=(j == 0), stop=(j == 7))
                nc.scalar.copy(dst_[:, m * 1024:(m + 1) * 1024], ps[:])
        for sc in range(8):  # v natural layout: out [s128, 256] per s-chunk
            ps = small_ps.tile([128, 512], F32, name="vps", tag="sps")
            for j in range(8):
                nc.tensor.matmul(
                    ps[:, 0:256],
                    xTr[:, j * 1024 + sc * 128: j * 1024 + (sc + 1) * 128],
                    wv_sb[:, j * 256:(j + 1) * 256],
                    start=(j == 0), stop=(j == 7))
            nc.scalar.copy(v_sb[:, sc * 256:(sc + 1) * 256], ps[:, 0:256])

    # ---- Phases C+D: attention, with idx(4..7) interleaved ----
    with tc.tile_pool(name="attn", bufs=1) as attn_pool, \
         tc.tile_pool(name="attn2", bufs=1) as attn2_pool:

        def emit_attn_head(tg, h):
            m, r = h // 2, (h % 2) * 64
            probT = attn_pool.tile([128, 8, 1024], F16, name="probT", tag="probT",
                                   bufs=2)
            for i in range(tg * 4, tg * 4 + 4):
                n_s = (i + 1) * 128
                ps = big_ps.tile([128, 1024], F32, name="aps", tag="mm")
                for grp in range((n_s + 511) // 512):
                    ns0, ns1 = grp * 512, min(n_s, (grp + 1) * 512)
                    nc.tensor.matmul(
                        ps[:, ns0:ns1],
                        qT[r:r + 64, m * 1024 + i * 128: m * 1024 + (i + 1) * 128],
                        kT[r:r + 64, m * 1024 + ns0: m * 1024 + ns1],
                        start=True, stop=False)
                    nc.tensor.matmul(ps[:, ns0:ns1], ident_b[:],
                                     mask_tiles[i][:, ns0:ns1],
                                     start=False, stop=True)
                if attn_no_sm:
                    continue
                scr = attn2_pool.tile([128, 1024], F16, name="scr", tag="scr", bufs=4)
                den = attn2_pool.tile([128, 1], F32, name="den", tag="den", bufs=4)
                nc.scalar.activation(scr[:, 0:n_s], ps[:, 0:n_s], AF.Exp,
                                     scale=0.125, accum_out=den[:])
                rec = attn2_pool.tile([128, 1], F32, name="rec", tag="rec", bufs=4)
                nc.vector.reciprocal(rec[:], den[:])
                prob = attn2_pool.tile([128, 1024], F16, name="prob", tag="prob",
                                       bufs=4)
                nc.scalar.activation(prob[:, 0:n_s], scr[:, 0:n_s], AF.Copy,
                                     scale=rec[:])
                if attn_no_tp:
                    continue
                tp = tp16_ps.tile([128, 1024], F16, name="tp", tag="tp16")
                for sc in range(i + 1):
                    nc.tensor.matmul(tp[:, sc * 128:(sc + 1) * 128],
                                     prob[:, sc * 128:(sc + 1) * 128], ident_h[:],
                                     is_transpose=True, start=True, stop=True)
                nc.vector.tensor_copy(
                    probT[:, 0:i + 1, i * 128:(i + 1) * 128],
                    tp[:, 0:n_s].rearrange("p (a b) -> p a b", b=128))
            if attn_no_sm or attn_no_tp:
                return
            pc = small_ps.tile([128, 512], F32, name="pc", tag="sps")
            n_sc = tg * 4 + 4
            for sc in range(n_sc):
                off = max(sc - tg * 4, 0) * 128
                nc.tensor.matmul(
                    pc[0:64, off:512],
                    v_sb[:, sc * 256 + h * 64: sc * 256 + (h + 1) * 64],
                    probT[:, sc, tg * 512 + off:(tg + 1) * 512],
                    start=(sc == 0), stop=(sc == n_sc - 1))
            ck, rr = h // 2, (h % 2) * 64
            nc.scalar.copy(ctxT[rr:rr + 64, ck * 1024 + tg * 512: ck * 1024 + (tg + 1) * 512],
                           pc[0:64, :])

        def emit_outproj(tg, dve_copy=False):
            for i in range(tg * 4, tg * 4 + 4):
                ps = big_ps.tile([128, 1024], F32, name="ops", tag="mm")
                for og in range(2):
                    for ck in range(2):
                        nc.tensor.matmul(
                            ps[:, og * 512:(og + 1) * 512],
                            ctxT[:, ck * 1024 + i * 128: ck * 1024 + (i + 1) * 128],
                            wo_sb[:, ck * 1024 + og * 512: ck * 1024 + (og + 1) * 512],
                            start=(ck == 0), stop=(ck == 1))
                out_sb = attn2_pool.tile([128, 1024], F32, name="out_sb", tag="out",
                                         bufs=2)
                if dve_copy:
                    nc.vector.tensor_copy(out_sb[:], ps[:])
                else:
                    nc.scalar.copy(out_sb[:], ps[:])
                nc.sync.dma_start(out_ap[i * 128:(i + 1) * 128, :], out_sb[:])

        if skip_attn or attn_no_sm or attn_no_tp:
            if not skip_attn:
                for h in range(HG):
                    emit_attn_head(0, h)
                    emit_idx(4 + h)
                for h in range(HG):
                    emit_attn_head(1, h)
            else:
                for h in range(HG):
                    emit_idx(4 + h)
            for i in range(NCHUNK):
                out_sb = attn2_pool.tile([128, 1024], F32, name="out_sb", tag="out",
                                         bufs=2)
                nc.vector.tensor_copy(out_sb[:], qT[:].bitcast(F32)[:, 0:1024])
                nc.sync.dma_start(out_ap[i * 128:(i + 1) * 128, :], out_sb[:])
        else:
            # topk(4..7) split into 4-round emission pieces so each attention
            # head's short DVE ops wait behind at most half a chunk's serial
            # top-k run in the in-order DVE queue
            emit_attn_head(0, 0)
            emit_idx_head(4, 4)
            emit_attn_head(0, 1)
            emit_idx_tail(4)
            emit_idx_head(5, 4)
            emit_attn_head(0, 2)
            emit_idx_tail(5)
            emit_idx_head(6, 4)
            emit_attn_head(0, 3)
            emit_idx_tail(6)
            emit_idx(7)
            emit_outproj(0)
            for h in range(HG):
                emit_attn_head(1, h)
            emit_outproj(1)

        if dbg is not None:
            def dump(name, ap):
                if name in dbg:
                    nc.sync.dma_start(dbg[name], ap.bitcast(dbg[name].dtype))
            dump("qiT", qiT[:])
            dump("kiw", kiw[0:68, :])
            dump("kiw2", kiw2[64:128, :])
            dump("w8", w8[:])
            dump("qT", qT[:])
            dump("kT", kT[:])
            dump("v", v_sb[:])
            dump("ctxT", ctxT[:])
            for i in range(NCHUNK):
                dump(f"mask{i}", mask_tiles[i][:])

    stack.close()


def _build_nc(loop=0):
    nc = bacc.Bacc("TRN2")
    x = nc.dram_tensor("x", [T, D], F32, kind="ExternalInput")
    wq = nc.dram_tensor("wq", [D, 256], F32, kind="ExternalInput")
    wk = nc.dram_tensor("wk", [D, 256], F32, kind="ExternalInput")
    wv = nc.dram_tensor("wv", [D, 256], F32, kind="ExternalInput")
    wo = nc.dram_tensor("wo", [256, D], F32, kind="ExternalInput")
    wi = nc.dram_tensor("wi", [D, 324], F32, kind="ExternalInput")
    out = nc.dram_tensor("out", [T, D], F32, kind="ExternalOutput")
    with tile.TileContext(nc) as tc:
        if loop:
            with tc.For_i(0, loop, 1):
                build_kernel(tc, out.ap(), x.ap(), wq.ap(), wk.ap(), wv.ap(), wo.ap(), wi.ap())
        else:
            build_kernel(tc, out.ap(), x.ap(), wq.ap(), wk.ap(), wv.ap(), wo.ap(), wi.ap())
    nc.compile()
    return nc


def kernel(x, Wq, Wk, Wv, Wo, bo, Wqi, Wki, Ww, _trace=False):
    _install_neff_cache()
    x, Wq, Wk, Wv, Wo, bo, Wqi, Wki, Ww = (
        np.ascontiguousarray(np.asarray(a, np.float32))
        for a in (x, Wq, Wk, Wv, Wo, bo, Wqi, Wki, Ww))
    nc = _build_nc()
    in_maps = _make_in_maps(x, Wq, Wk, Wv, Wo, Wqi, Wki, Ww)
    res = run_bass_kernel_spmd(nc, in_maps, core_ids=list(range(8)), trace=_trace)
    outs = [r["out"] for r in res.results]
    full = np.stack([sum(outs[b * 4:(b + 1) * 4]) + bo for b in range(B)], axis=0)
    full = full.astype(np.float32)
    if _trace:
        return full, res
    return full


def _make_in_maps(x, Wq, Wk, Wv, Wo, Wqi, Wki, Ww):
    wi = np.ascontiguousarray(np.concatenate([Wqi, Wki, Ww], axis=1))
    in_maps = []
    for b in range(B):
        for g in range(4):
            c = slice(g * 256, (g + 1) * 256)
            in_maps.append({
                "x": np.ascontiguousarray(x[b]),
                "wq": np.ascontiguousarray(Wq[:, c]),
                "wk": np.ascontiguousarray(Wk[:, c]),
                "wv": np.ascontiguousarray(Wv[:, c]),
                "wo": np.ascontiguousarray(Wo[c, :]),
                "wi": wi,
            })
    return in_maps


def bench_exec_ns(inputs, iters=10, loop=256):
    """Per-iteration device time: the kernel body loops `loop` times inside one
    NEFF; dispatch-overhead floor (loop=1 variant) is subtracted via the slope
    between two loop counts. Returns ns per kernel iteration."""
    lo = max(1, loop // 8)
    t_hi = _bench_exec_wall(inputs, iters, loop)
    t_lo = _bench_exec_wall(inputs, iters, lo)
    return (t_hi - t_lo) / (loop - lo) * 1e9


def _bench_exec_wall(inputs, iters, loop):
    import time

    import jax
    from jax.experimental.shard_map import shard_map
    from jax.sharding import Mesh, NamedSharding, PartitionSpec

    import concourse.bass2jax as b2j

    _install_neff_cache()
    b2j.install_neuronx_cc_hook()
    nc = _build_nc(loop=loop)
    ins = {k: np.ascontiguousarray(np.asarray(v, np.float32)) for k, v in inputs.items()}
    in_maps = _make_in_maps(ins["x"], ins["Wq"], ins["Wk"], ins["Wv"], ins["Wo"],
                            ins["Wqi"], ins["Wki"], ins["Ww"])

    partition_name = nc.partition_id_tensor.name if nc.partition_id_tensor else None
    in_names, out_names, out_avals, zero_outs = [], [], [], []
    for alloc in nc.m.functions[0].allocations:
        if not isinstance(alloc, mybir.MemoryLocationSet):
            continue
        name = alloc.memorylocations[0].name
        if alloc.kind == "ExternalInput":
            if name != partition_name:
                in_names.append(name)
        elif alloc.kind == "ExternalOutput":
            shape = tuple(alloc.tensor_shape)
            dtype = mybir.dt.np(alloc.dtype)
            out_names.append(name)
            out_avals.append(jax.core.ShapedArray(shape, dtype))
            zero_outs.append(np.zeros(shape, dtype))
    n_params = len(in_names)
    all_in_names = list(in_names) + list(out_names)
    if partition_name is not None:
        all_in_names.append(partition_name)

    def _body(*args):
        operands = list(args)
        if partition_name is not None:
            operands.append(b2j.partition_id_tensor())
        outs = b2j._bass_exec_p.bind(
            *operands,
            out_avals=tuple(out_avals),
            in_names=tuple(all_in_names),
            out_names=tuple(out_names),
            lowering_input_output_aliases=(),
            sim_require_finite=True,
            sim_require_nnan=True,
            nc=nc,
        )
        return tuple(outs)

    n_cores = len(in_maps)
    devices = jax.devices()[:n_cores]
    mesh = Mesh(np.asarray(devices), ("core",))
    in_specs = (PartitionSpec("core"),) * (n_params + len(out_names))
    out_specs = (PartitionSpec("core"),) * len(out_names)
    fn = jax.jit(shard_map(_body, mesh=mesh, in_specs=in_specs,
                           out_specs=out_specs, check_rep=False))
    sharding = NamedSharding(mesh, PartitionSpec("core"))
    dev_args = [
        jax.device_put(
            np.concatenate([np.asarray(in_maps[c][nm]) for c in range(n_cores)], axis=0),
            sharding)
        for nm in in_names
    ] + [
        jax.device_put(np.concatenate([z] * n_cores, axis=0), sharding)
        for z in zero_outs
    ]
    r = fn(*dev_args)
    jax.block_until_ready(r)
    times = []
    for _ in range(iters):
        t0 = time.perf_counter()
        r = fn(*dev_args)
        jax.block_until_ready(r)
        times.append(time.perf_counter() - t0)
    return min(times)


if __name__ == "__main__":
    rng = np.random.default_rng(0)
    ins = {
        "x": rng.standard_normal((B, T, D)).astype(np.float32),
        "Wq": (rng.standard_normal((D, D)) * 0.02).astype(np.float32),
        "Wk": (rng.standard_normal((D, D)) * 0.02).astype(np.float32),
        "Wv": (rng.standard_normal((D, D)) * 0.02).astype(np.float32),
        "Wo": (rng.standard_normal((D, D)) * 0.02).astype(np.float32),
        "bo": np.zeros(D, np.float32),
        "Wqi": (rng.standard_normal((D, HI * IHD)) * 0.02).astype(np.float32),
        "Wki": (rng.standard_normal((D, IHD)) * 0.02).astype(np.float32),
        "Ww": (rng.standard_normal((D, HI)) * 0.02).astype(np.float32),
    }
    out = kernel(**ins)
    print("out", out.shape, out.dtype, float(np.abs(out).max()))


# revision 65
# speedup vs baseline: 1.0481x; 1.0046x over previous
"""Bass/Trainium2 kernel for MultiHeadAttentionWithDSA (sparse attention with
lightning-indexer top-64 key selection), sharded over 8 NeuronCores.

Sharding: core = b*4 + g  (b in {0,1} batch, g in {0..3} head-group of 4 heads).
Each core computes a partial output  ctx_g @ Wo[g*256:(g+1)*256, :]  for its
batch; the host sums the 4 partials per batch and adds the bias.

Perf notes (measured on hw, baseline 490us -> 287us):
- The indexer path (x^T, qi/ki projections, idx scores) stays true fp32: with
  fp32r the hw top-64 selection drifts from the fp32 reference at score-gap
  boundaries (42 rows > 1e-2, rel err 2.8e-2 > the 2e-2 gate).
- q/k/v/out-proj matmuls run fp32r (1 cycle/row); probs and v are fp16.
- The top-k additive mask is accumulated into the score PSUM with a
  bf16-identity matmul instead of a DVE tensor add.
- softmax probs are normalized on the Act engine (Copy activation with a
  per-partition reciprocal scale). gpsimd/Pool TensorScalarPtr is a Q7
  software op at ~6.8us per call and single-handedly cost the old kernel
  ~220us -- never put per-element work on gpsimd here.
- prob transposes are fp16 PE transposes (1 cycle/row) into fp16 PSUM, copied
  by DVE (2x_1p mode). DmaTransposeAnt is NOT usable for blocked 3D outputs:
  walrus lowers it differently from CoreSim's semantic model (it even stomps
  unrelated SBUF), and its fixed +16 DMA semaphores break Tile's wait
  accounting when >1 transpose feeds one consumer.
- Weight loads are one batched DMA each, straight into fp32r tiles (bitcast
  DRAM APs); x is loaded in 4 chunked DMAs and transposed per token-half so
  the indexer projections + chunk-0..3 top-k (serial DVE chain, the critical
  resource) start while the second half of x is still in flight.
- Emission interleaves idx chunks 4..7 with the first token-group's attention
  heads, with each chunk's 8 top-k rounds SPLIT across two heads so the
  attention chain's short DVE ops (recips, probT copies) wait behind at most
  ~4 serial top-k rounds in the in-order DVE queue. Weight DMAs are emitted
  after the first x-chunk DMA (x feeds the critical-path transposes).
  Tried and measured WORSE than this arrangement: two unsplit idx chunks
  after the first head (305us), all four idx chunks after all of tg0's heads
  (299us AND wrong results -- a latent sync bug surfaces under that order),
  normalize on DVE instead of Act (296us), normalize fused into the transpose
  as a diag(1/den) matmul (289us), xTr copies on Act (301us).
"""

import numpy as np

import concourse.bacc as bacc
import concourse.bass as bass
import concourse.mybir as mybir
import concourse.tile as tile
from concourse import masks
from concourse.bass_utils import run_bass_kernel_spmd

F32 = mybir.dt.float32
F32R = mybir.dt.float32r
F16 = mybir.dt.float16
BF16 = mybir.dt.bfloat16
AF = mybir.ActivationFunctionType
ALU = mybir.AluOpType

B, T, D = 2, 1024, 1024
H, HD = 16, 64          # total heads, head dim
HG = 4                  # heads per core
HI, IHD = 4, 64         # index heads, index head dim
TOPK = 64
NCHUNK = T // 128       # 8 token chunks of 128
NEG = -3.0e30           # causal-invalid marker (additive mask value)
SENT = -1.0e30          # match_replace sentinel (distinct from NEG)

_NEFF_CACHE = "/var/tmp/bass-neff-cache"


def _install_neff_cache():
    """walrus compile output cache keyed on BIR hash (compiles are minutes)."""
    import hashlib
    import os
    import shutil

    import concourse.bass2jax as b2j

    if getattr(b2j, "_dsa_neff_cache_installed", False):
        return
    orig = b2j.compile_bir_kernel

    def cached(bir_json, tmpdir, neff_name="file.neff"):
        try:
            h = hashlib.sha256(
                bir_json if isinstance(bir_json, bytes) else bir_json.encode()
            ).hexdigest()[:24]
            os.makedirs(_NEFF_CACHE, exist_ok=True)
            hit = os.path.join(_NEFF_CACHE, h + ".neff")
            if os.path.exists(hit):
                dst = os.path.join(tmpdir, neff_name)
                shutil.copyfile(hit, dst)
                return dst
            neff = orig(bir_json, tmpdir, neff_name)
            shutil.copyfile(neff, hit + ".tmp")
            os.replace(hit + ".tmp", hit)
            return neff
        except OSError:
            return orig(bir_json, tmpdir, neff_name)

    b2j.compile_bir_kernel = cached
    b2j._dsa_neff_cache_installed = True


def R(ap):
    return ap.bitcast(F32R)


def build_kernel(tc, out_ap, x_ap, wq_ap, wk_ap, wv_ap, wo_ap, wi_ap, dbg=None):
    """Emit the per-core kernel. All APs are DRAM tensors:
    x [1024,1024], wq/wk/wv [1024,256], wo [256,1024],
    wi [1024,324] = concat(Wqi[1024,256], Wki[1024,64], Ww[1024,4]).
    out [1024,1024] partial (pre-bias, pre-reduction over head groups).
    """
    nc = tc.nc
    import os
    from contextlib import ExitStack
    skip_attn = os.environ.get("K_SKIP_ATTN") == "1"
    skip_topk = os.environ.get("K_SKIP_TOPK") == "1"
    attn_no_sm = os.environ.get("K_ATTN_NO_SM") == "1"   # scores+mask only
    attn_no_tp = os.environ.get("K_ATTN_NO_TP") == "1"   # + exp/recip/normalize
    stack = ExitStack()

    const_pool = stack.enter_context(tc.tile_pool(name="const", bufs=1))
    ident_b = const_pool.tile([128, 128], BF16)
    masks.make_identity(nc, ident_b[:])
    ident_h = const_pool.tile([128, 128], F16)
    masks.make_identity(nc, ident_h[:])
    ident_f = const_pool.tile([128, 128], F32)
    masks.make_identity(nc, ident_f[:])
    causal = const_pool.tile([128, 128], F32)
    masks.make_causal_mask(nc, causal[:], mask_val=NEG)

    # ---- weights: one DMA each, straight into f32r tiles ----
    w_pool = stack.enter_context(tc.tile_pool(name="weights", bufs=1))
    wq_sb = w_pool.tile([128, 8 * 256], F32R)
    wk_sb = w_pool.tile([128, 8 * 256], F32R)
    wv_sb = w_pool.tile([128, 8 * 256], F32R)
    wo_sb = w_pool.tile([128, 2 * 1024], F32R)
    wi_sb = w_pool.tile([128, 8 * 324], F32)

    def emit_wi_dma():
        # wi feeds B1 (~15us in); the x chunk DMAs ahead of it feed the
        # critical-path transposes
        nc.sync.dma_start(
            wi_sb[:].rearrange("p (j c) -> p j c", c=324),
            wi_ap.rearrange("(j p) c -> p j c", p=128))

    def emit_qkvo_dmas():
        # wq/wk/wv have slack until B2: emitted after ALL x-chunk DMAs so
        # they never delay the transpose pipeline. wo (needed only by
        # outproj, ~100us in) is deferred further still.
        for ap_, dst_, c in ((wq_ap, wq_sb, 256), (wk_ap, wk_sb, 256),
                             (wv_ap, wv_sb, 256)):
            nc.sync.dma_start(
                dst_[:].rearrange("p (j c) -> p j c", c=c),
                R(ap_.rearrange("(j p) c -> p j c", p=128)))

    def emit_wo_dma():
        nc.sync.dma_start(
            wo_sb[:].rearrange("p (j c) -> p j c", c=1024),
            R(wo_ap.rearrange("(j p) c -> p j c", p=128)))

    act_pool = stack.enter_context(tc.tile_pool(name="acts", bufs=1))
    qT = act_pool.tile([128, 2 * 1024], F32R)    # heads (2m,2m+1) rows, tokens free
    kT = act_pool.tile([128, 2 * 1024], F32R)
    qiT = act_pool.tile([128, 2 * 1024], F32)
    kiw = act_pool.tile([128, 1024], F32)        # rows 0-63 kiT, 64-67 wT logits
    kiw2 = act_pool.tile([128, 1024], F32)       # rows 64-127: copy of kiT (odd index heads)
    v_sb = act_pool.tile([128, 8 * 256], F16)    # [s-chunk sc] at cols sc*256, head cols inside
    ctxT = act_pool.tile([128, 2 * 1024], F32R)  # [ck] at cols ck*1024
    w8 = act_pool.tile([128, 32], F32)           # softmax(x@Ww)/8, chunk i at cols 4i
    mask_tiles = [act_pool.tile([128, (i + 1) * 128], BF16, name=f"mask{i}",
                                tag=f"mask{i}") for i in range(NCHUNK)]

    idx_pool = stack.enter_context(tc.tile_pool(name="idx", bufs=1))

    big_ps = stack.enter_context(tc.tile_pool(name="big_ps", bufs=2, space="PSUM"))
    small_ps = stack.enter_context(tc.tile_pool(name="small_ps", bufs=2, space="PSUM"))
    tp16_ps = stack.enter_context(tc.tile_pool(name="tp16_ps", bufs=2, space="PSUM"))

    idx_state = {}

    def emit_idx_head(i, head_rounds):
        n_s = (i + 1) * 128
        if skip_topk:
            return
        work = idx_pool.tile([128, 1024], F32, name="work", tag="work", bufs=4)
        for h in range(HI):
            m, r = h // 2, (h % 2) * 64
            ps = big_ps.tile([128, 1024], F32, name="ips", tag="mm")
            for grp in range((n_s + 511) // 512):
                ns0, ns1 = grp * 512, min(n_s, (grp + 1) * 512)
                ki_rhs = kiw[0:64, ns0:ns1] if r == 0 else kiw2[64:128, ns0:ns1]
                nc.tensor.matmul(
                    ps[:, ns0:ns1],
                    qiT[r:r + 64, m * 1024 + i * 128: m * 1024 + (i + 1) * 128],
                    ki_rhs, start=True, stop=True)
            if h == 0:
                nc.scalar.activation(work[:, 0:n_s], ps[:, 0:n_s], AF.Relu,
                                     scale=w8[:, i * 4 + h: i * 4 + h + 1])
            else:
                aw = idx_pool.tile([128, 1024], F32, name="aw", tag="aw", bufs=3)
                nc.scalar.activation(aw[:, 0:n_s], ps[:, 0:n_s], AF.Relu,
                                     scale=w8[:, i * 4 + h: i * 4 + h + 1])
                nc.gpsimd.tensor_tensor(work[:, 0:n_s], work[:, 0:n_s],
                                        aw[:, 0:n_s], op=ALU.add)
        nc.gpsimd.tensor_tensor(work[:, i * 128:(i + 1) * 128],
                                work[:, i * 128:(i + 1) * 128], causal[:], op=ALU.add)
        tmax = idx_pool.tile([128, 8], F32, name="tmax", tag="tmax", bufs=2)
        for _ in range(head_rounds):
            nc.vector.max(tmax[:], work[:, 0:n_s])
            nc.vector.match_replace(work[:, 0:n_s], tmax[:], work[:, 0:n_s], SENT)
        idx_state[i] = (work, tmax, n_s, head_rounds)

    def emit_idx_tail(i):
        if skip_topk:
            mk = mask_tiles[i]
            nc.gpsimd.memset(mk[:], 0.0)
            nc.gpsimd.tensor_tensor(mk[:, i * 128:(i + 1) * 128],
                                    mk[:, i * 128:(i + 1) * 128], causal[:], op=ALU.add)
            return
        work, tmax, n_s, head_rounds = idx_state.pop(i)
        for _ in range(8 - head_rounds):
            nc.vector.max(tmax[:], work[:, 0:n_s])
            nc.vector.match_replace(work[:, 0:n_s], tmax[:], work[:, 0:n_s], SENT)
        mk = mask_tiles[i]
        nc.vector.tensor_scalar(mk[:], work[:, 0:n_s], SENT, NEG,
                                op0=ALU.not_equal, op1=ALU.mult)
        nc.gpsimd.tensor_tensor(mk[:, i * 128:(i + 1) * 128],
                                mk[:, i * 128:(i + 1) * 128], causal[:], op=ALU.add)

    def emit_idx(i):
        emit_idx_head(i, 8)
        emit_idx_tail(i)

    with tc.tile_pool(name="xscope", bufs=1) as xscope, \
         tc.tile_pool(name="xtok", bufs=2) as xtok_pool:
        xT = xscope.tile([128, 8 * 1024], F32)   # [d-chunk j] at cols j*1024, feature-major
        xTr = xscope.tile([128, 8 * 1024], F32R)  # rounded shadow for f32r matmuls
        xT3 = xT[:].rearrange("p (j c) -> p j c", c=1024)
        xTr3 = xTr[:].rearrange("p (j c) -> p j c", c=1024)

        def emit_b1_half(tg):
            """Indexer projections for token-half tg (needs only xT token cols
            tg*512..(tg+1)*512, i.e. token chunks tg*4..tg*4+3 transposed)."""
            for m in range(2):  # qiT
                ps = small_ps.tile([128, 512], F32, name="b1", tag="sps")
                for j in range(8):
                    nc.tensor.matmul(
                        ps[:],
                        wi_sb[:, j * 324 + m * 128: j * 324 + (m + 1) * 128],
                        xT[:, j * 1024 + tg * 512: j * 1024 + (tg + 1) * 512],
                        start=(j == 0), stop=(j == 7))
                nc.scalar.copy(qiT[:, m * 1024 + tg * 512: m * 1024 + (tg + 1) * 512],
                               ps[:])
            ps = small_ps.tile([128, 512], F32, name="b1k", tag="sps")
            for j in range(8):  # kiT + wT logits (68 cols of wi)
                nc.tensor.matmul(
                    ps[0:68, :],
                    wi_sb[:, j * 324 + 256: j * 324 + 324],
                    xT[:, j * 1024 + tg * 512: j * 1024 + (tg + 1) * 512],
                    start=(j == 0), stop=(j == 7))
            nc.scalar.copy(kiw[0:68, tg * 512:(tg + 1) * 512], ps[0:68, :])
            nc.gpsimd.dma_start(kiw2[64:128, tg * 512:(tg + 1) * 512],
                                kiw[0:64, tg * 512:(tg + 1) * 512])
            for i in range(tg * 4, tg * 4 + 4):
                # w softmax per chunk: transpose wT logits [4, t128] -> [t128, 4]
                pw = small_ps.tile([128, 512], F32, name="pw", tag="sps")
                nc.tensor.transpose(pw[:, 0:4], kiw[64:68, i * 128:(i + 1) * 128],
                                    ident_f[64:68, 64:68])
                wexp = act_pool.tile([128, 4], F32, name="wexp", tag="wexp", bufs=2)
                wden = act_pool.tile([128, 1], F32, name="wden", tag="wden", bufs=2)
                nc.scalar.activation(wexp[:], pw[:, 0:4], AF.Exp, accum_out=wden[:])
                wrec = act_pool.tile([128, 1], F32, name="wrec", tag="wrec", bufs=2)
                nc.vector.reciprocal(wrec[:], wden[:])
                nc.vector.tensor_scalar(w8[:, i * 4:(i + 1) * 4], wexp[:], wrec[:],
                                        0.125, op0=ALU.mult, op1=ALU.mult)

        # ---- Phase A + B1, pipelined by token-half ----
        for half in range(2):
            for hq in range(2):
                xt = xtok_pool.tile([128, 2 * 1024], F32, name="xt", tag="xtok")
                nc.sync.dma_start(
                    xt[:].rearrange("p (i c) -> p i c", c=1024),
                    x_ap.rearrange("(i p) c -> p i c", p=128)[
                        :, half * 4 + hq * 2: half * 4 + (hq + 1) * 2, :])
                if half == 0 and hq == 1:
                    emit_wi_dma()
                if half == 1 and hq == 1:
                    emit_qkvo_dmas()
                for q in range(2):
                    i = half * 4 + hq * 2 + q
                    pt = big_ps.tile([128, 1024], F32, name="pt", tag="mm")
                    for j in range(8):
                        nc.tensor.matmul(
                            pt[:, j * 128:(j + 1) * 128],
                            xt[:, q * 1024 + j * 128: q * 1024 + (j + 1) * 128],
                            ident_f[:], is_transpose=True, start=True, stop=True)
                    nc.scalar.copy(xT3[:, :, i * 128:(i + 1) * 128],
                                   pt[:].rearrange("p (j c) -> p j c", c=128))
            # gpsimd-issued casting DMA: keeps the f32r shadow copy off the
            # DVE queue (where it would serialize behind the top-k runs and
            # gate B2 -> attention start)
            nc.gpsimd.dma_start(xTr3[:, :, half * 512:(half + 1) * 512],
                                xT3[:, :, half * 512:(half + 1) * 512])
            emit_b1_half(half)
            if half == 0:
                for i in range(4):
                    emit_idx(i)

        emit_wo_dma()
        # ---- Phase B2: q/k/v projections ----
        for (wsb, dst_) in ((wq_sb, qT), (wk_sb, kT)):
            for m in range(2):
                ps = big_ps.tile([128, 1024], F32, name="qps", tag="mm")
                for tg in range(2):
                    for j in range(8):
                        nc.tensor.matmul(
                            ps[:, tg * 512:(tg + 1) * 512],
                            wsb[:, j * 256 + m * 128: j * 256 + (m + 1) * 128],
                            xTr[:, j * 1024 + tg * 512: j * 1024 + (tg + 1) * 512],
                            start=(j == 0), stop=(j == 7))
                nc.scalar.copy(dst_[:, m * 1024:(m + 1) * 1024], ps[:])
        for sc in range(8):  # v natural layout: out [s128, 256] per s-chunk
            ps = small_ps.tile([128, 512], F32, name="vps", tag="sps")
            for j in range(8):
                nc.tensor.matmul(
                    ps[:, 0:256],
                    xTr[:, j * 1024 + sc * 128: j * 1024 + (sc + 1) * 128],
                    wv_sb[:, j * 256:(j + 1) * 256],
                    start=(j == 0), stop=(j == 7))
            nc.scalar.copy(v_sb[:, sc * 256:(sc + 1) * 256], ps[:, 0:256])

    # ---- Phases C+D: attention, with idx(4..7) interleaved ----
    with tc.tile_pool(name="attn", bufs=1) as attn_pool, \
         tc.tile_pool(name="attn2", bufs=1) as attn2_pool:

        def emit_attn_head(tg, h):
            m, r = h // 2, (h % 2) * 64
            probT = attn_pool.tile([128, 8, 1024], F16, name="probT", tag="probT",
                                   bufs=2)
            for i in range(tg * 4, tg * 4 + 4):
                n_s = (i + 1) * 128
                ps = big_ps.tile([128, 1024], F32, name="aps", tag="mm")
                for grp in range((n_s + 511) // 512):
                    ns0, ns1 = grp * 512, min(n_s, (grp + 1) * 512)
                    nc.tensor.matmul(
                        ps[:, ns0:ns1],
                        qT[r:r + 64, m * 1024 + i * 128: m * 1024 + (i + 1) * 128],
                        kT[r:r + 64, m * 1024 + ns0: m * 1024 + ns1],
                        start=True, stop=False)
                    nc.tensor.matmul(ps[:, ns0:ns1], ident_b[:],
                                     mask_tiles[i][:, ns0:ns1],
                                     start=False, stop=True)
                if attn_no_sm:
                    continue
                scr = attn2_pool.tile([128, 1024], F16, name="scr", tag="scr", bufs=4)
                den = attn2_pool.tile([128, 1], F32, name="den", tag="den", bufs=4)
                nc.scalar.activation(scr[:, 0:n_s], ps[:, 0:n_s], AF.Exp,
                                     scale=0.125, accum_out=den[:])
                rec = attn2_pool.tile([128, 1], F32, name="rec", tag="rec", bufs=4)
                nc.vector.reciprocal(rec[:], den[:])
                prob = attn2_pool.tile([128, 1024], F16, name="prob", tag="prob",
                                       bufs=4)
                nc.scalar.activation(prob[:, 0:n_s], scr[:, 0:n_s], AF.Copy,
                                     scale=rec[:])
                if attn_no_tp:
                    continue
                tp = tp16_ps.tile([128, 1024], F16, name="tp", tag="tp16")
                for sc in range(i + 1):
                    nc.tensor.matmul(tp[:, sc * 128:(sc + 1) * 128],
                                     prob[:, sc * 128:(sc + 1) * 128], ident_h[:],
                                     is_transpose=True, start=True, stop=True)
                nc.vector.tensor_copy(
                    probT[:, 0:i + 1, i * 128:(i + 1) * 128],
                    tp[:, 0:n_s].rearrange("p (a b) -> p a b", b=128))
            if attn_no_sm or attn_no_tp:
                return
            pc = small_ps.tile([128, 512], F32, name="pc", tag="sps")
            n_sc = tg * 4 + 4
            for sc in range(n_sc):
                off = max(sc - tg * 4, 0) * 128
                nc.tensor.matmul(
                    pc[0:64, off:512],
                    v_sb[:, sc * 256 + h * 64: sc * 256 + (h + 1) * 64],
                    probT[:, sc, tg * 512 + off:(tg + 1) * 512],
                    start=(sc == 0), stop=(sc == n_sc - 1))
            ck, rr = h // 2, (h % 2) * 64
            nc.scalar.copy(ctxT[rr:rr + 64, ck * 1024 + tg * 512: ck * 1024 + (tg + 1) * 512],
                           pc[0:64, :])

        def emit_outproj(tg, dve_copy=False):
            for i in range(tg * 4, tg * 4 + 4):
                ps = big_ps.tile([128, 1024], F32, name="ops", tag="mm")
                for og in range(2):
                    for ck in range(2):
                        nc.tensor.matmul(
                            ps[:, og * 512:(og + 1) * 512],
                            ctxT[:, ck * 1024 + i * 128: ck * 1024 + (i + 1) * 128],
                            wo_sb[:, ck * 1024 + og * 512: ck * 1024 + (og + 1) * 512],
                            start=(ck == 0), stop=(ck == 1))
                out_sb = attn2_pool.tile([128, 1024], F32, name="out_sb", tag="out",
                                         bufs=2)
                if dve_copy:
                    nc.vector.tensor_copy(out_sb[:], ps[:])
                else:
                    nc.scalar.copy(out_sb[:], ps[:])
                nc.sync.dma_start(out_ap[i * 128:(i + 1) * 128, :], out_sb[:])

        if skip_attn or attn_no_sm or attn_no_tp:
            if not skip_attn:
                for h in range(HG):
                    emit_attn_head(0, h)
                    emit_idx(4 + h)
                for h in range(HG):
                    emit_attn_head(1, h)
            else:
                for h in range(HG):
                    emit_idx(4 + h)
            for i in range(NCHUNK):
                out_sb = attn2_pool.tile([128, 1024], F32, name="out_sb", tag="out",
                                         bufs=2)
                nc.vector.tensor_copy(out_sb[:], qT[:].bitcast(F32)[:, 0:1024])
                nc.sync.dma_start(out_ap[i * 128:(i + 1) * 128, :], out_sb[:])
        else:
            # topk(4..7) split into 4-round emission pieces so each attention
            # head's short DVE ops wait behind at most half a chunk's serial
            # top-k run in the in-order DVE queue
            emit_attn_head(0, 0)
            emit_idx_head(4, 4)
            emit_attn_head(0, 1)
            emit_idx_tail(4)
            emit_idx_head(5, 4)
            emit_attn_head(0, 2)
            emit_idx_tail(5)
            emit_idx_head(6, 4)
            emit_attn_head(0, 3)
            emit_idx_tail(6)
            emit_idx(7)
            emit_outproj(0)
            for h in range(HG):
                emit_attn_head(1, h)
            emit_outproj(1)

        if dbg is not None:
            def dump(name, ap):
                if name in dbg:
                    nc.sync.dma_start(dbg[name], ap.bitcast(dbg[name].dtype))
            dump("qiT", qiT[:])
            dump("kiw", kiw[0:68, :])
            dump("kiw2", kiw2[64:128, :])
            dump("w8", w8[:])
            dump("qT", qT[:])
            dump("kT", kT[:])
            dump("v", v_sb[:])
            dump("ctxT", ctxT[:])
            for i in range(NCHUNK):
                dump(f"mask{i}", mask_tiles[i][:])

    stack.close()


def _build_nc(loop=0):
    nc = bacc.Bacc("TRN2")
    x = nc.dram_tensor("x", [T, D], F32, kind="ExternalInput")
    wq = nc.dram_tensor("wq", [D, 256], F32, kind="ExternalInput")
    wk = nc.dram_tensor("wk", [D, 256], F32, kind="ExternalInput")
    wv = nc.dram_tensor("wv", [D, 256], F32, kind="ExternalInput")
    wo = nc.dram_tensor("wo", [256, D], F32, kind="ExternalInput")
    wi = nc.dram_tensor("wi", [D, 324], F32, kind="ExternalInput")
    out = nc.dram_tensor("out", [T, D], F32, kind="ExternalOutput")
    with tile.TileContext(nc) as tc:
        if loop:
            with tc.For_i(0, loop, 1):
                build_kernel(tc, out.ap(), x.ap(), wq.ap(), wk.ap(), wv.ap(), wo.ap(), wi.ap())
        else:
            build_kernel(tc, out.ap(), x.ap(), wq.ap(), wk.ap(), wv.ap(), wo.ap(), wi.ap())
    nc.compile()
    return nc


def kernel(x, Wq, Wk, Wv, Wo, bo, Wqi, Wki, Ww, _trace=False):
    _install_neff_cache()
    x, Wq, Wk, Wv, Wo, bo, Wqi, Wki, Ww = (
        np.ascontiguousarray(np.asarray(a, np.float32))
        for a in (x, Wq, Wk, Wv, Wo, bo, Wqi, Wki, Ww))
    nc = _build_nc()
    in_maps = _make_in_maps(x, Wq, Wk, Wv, Wo, Wqi, Wki, Ww)
    res = run_bass_kernel_spmd(nc, in_maps, core_ids=list(range(8)), trace=_trace)
    outs = [r["out"] for r in res.results]
    full = np.stack([sum(outs[b * 4:(b + 1) * 4]) + bo for b in range(B)], axis=0)
    full = full.astype(np.float32)
    if _trace:
        return full, res
    return full


def _make_in_maps(x, Wq, Wk, Wv, Wo, Wqi, Wki, Ww):
    wi = np.ascontiguousarray(np.concatenate([Wqi, Wki, Ww], axis=1))
    in_maps = []
    for b in range(B):
        for g in range(4):
            c = slice(g * 256, (g + 1) * 256)
            in_maps.append({
                "x": np.ascontiguousarray(x[b]),
                "wq": np.ascontiguousarray(Wq[:, c]),
                "wk": np.ascontiguousarray(Wk[:, c]),
                "wv": np.ascontiguousarray(Wv[:, c]),
                "wo": np.ascontiguousarray(Wo[c, :]),
                "wi": wi,
            })
    return in_maps


def bench_exec_ns(inputs, iters=10, loop=256):
    """Per-iteration device time: the kernel body loops `loop` times inside one
    NEFF; dispatch-overhead floor (loop=1 variant) is subtracted via the slope
    between two loop counts. Returns ns per kernel iteration."""
    lo = max(1, loop // 8)
    t_hi = _bench_exec_wall(inputs, iters, loop)
    t_lo = _bench_exec_wall(inputs, iters, lo)
    return (t_hi - t_lo) / (loop - lo) * 1e9


def _bench_exec_wall(inputs, iters, loop):
    import time

    import jax
    from jax.experimental.shard_map import shard_map
    from jax.sharding import Mesh, NamedSharding, PartitionSpec

    import concourse.bass2jax as b2j

    _install_neff_cache()
    b2j.install_neuronx_cc_hook()
    nc = _build_nc(loop=loop)
    ins = {k: np.ascontiguousarray(np.asarray(v, np.float32)) for k, v in inputs.items()}
    in_maps = _make_in_maps(ins["x"], ins["Wq"], ins["Wk"], ins["Wv"], ins["Wo"],
                            ins["Wqi"], ins["Wki"], ins["Ww"])

    partition_name = nc.partition_id_tensor.name if nc.partition_id_tensor else None
    in_names, out_names, out_avals, zero_outs = [], [], [], []
    for alloc in nc.m.functions[0].allocations:
        if not isinstance(alloc, mybir.MemoryLocationSet):
            continue
        name = alloc.memorylocations[0].name
        if alloc.kind == "ExternalInput":
            if name != partition_name:
                in_names.append(name)
        elif alloc.kind == "ExternalOutput":
            shape = tuple(alloc.tensor_shape)
            dtype = mybir.dt.np(alloc.dtype)
            out_names.append(name)
            out_avals.append(jax.core.ShapedArray(shape, dtype))
            zero_outs.append(np.zeros(shape, dtype))
    n_params = len(in_names)
    all_in_names = list(in_names) + list(out_names)
    if partition_name is not None:
        all_in_names.append(partition_name)

    def _body(*args):
        operands = list(args)
        if partition_name is not None:
            operands.append(b2j.partition_id_tensor())
        outs = b2j._bass_exec_p.bind(
            *operands,
            out_avals=tuple(out_avals),
            in_names=tuple(all_in_names),
            out_names=tuple(out_names),
            lowering_input_output_aliases=(),
            sim_require_finite=True,
            sim_require_nnan=True,
            nc=nc,
        )
        return tuple(outs)

    n_cores = len(in_maps)
    devices = jax.devices()[:n_cores]
    mesh = Mesh(np.asarray(devices), ("core",))
    in_specs = (PartitionSpec("core"),) * (n_params + len(out_names))
    out_specs = (PartitionSpec("core"),) * len(out_names)
    fn = jax.jit(shard_map(_body, mesh=mesh, in_specs=in_specs,
                           out_specs=out_specs, check_rep=False))
    sharding = NamedSharding(mesh, PartitionSpec("core"))
    dev_args = [
        jax.device_put(
            np.concatenate([np.asarray(in_maps[c][nm]) for c in range(n_cores)], axis=0),
            sharding)
        for nm in in_names
    ] + [
        jax.device_put(np.concatenate([z] * n_cores, axis=0), sharding)
        for z in zero_outs
    ]
    r = fn(*dev_args)
    jax.block_until_ready(r)
    times = []
    for _ in range(iters):
        t0 = time.perf_counter()
        r = fn(*dev_args)
        jax.block_until_ready(r)
        times.append(time.perf_counter() - t0)
    return min(times)


if __name__ == "__main__":
    rng = np.random.default_rng(0)
    ins = {
        "x": rng.standard_normal((B, T, D)).astype(np.float32),
        "Wq": (rng.standard_normal((D, D)) * 0.02).astype(np.float32),
        "Wk": (rng.standard_normal((D, D)) * 0.02).astype(np.float32),
        "Wv": (rng.standard_normal((D, D)) * 0.02).astype(np.float32),
        "Wo": (rng.standard_normal((D, D)) * 0.02).astype(np.float32),
        "bo": np.zeros(D, np.float32),
        "Wqi": (rng.standard_normal((D, HI * IHD)) * 0.02).astype(np.float32),
        "Wki": (rng.standard_normal((D, IHD)) * 0.02).astype(np.float32),
        "Ww": (rng.standard_normal((D, HI)) * 0.02).astype(np.float32),
    }
    out = kernel(**ins)
    print("out", out.shape, out.dtype, float(np.abs(out).max()))
